# revision 26
# baseline (speedup 1.0000x reference)
"""Multi-head causal self-attention (B=2, T=2048, C=768, H=12, D=64) on 8
Trainium2 NeuronCores.

Sharding: 24 (batch, head) units -> 3 heads per core; cores 0-3 take batch 0,
cores 4-7 take batch 1. Each core computes q/k/v projections for its 3 heads,
flash-style causal attention fully on-chip (no T x T tensor ever touches HBM),
and a partial output projection with its 192-row slice of Wproj. The host sums
the 4 partial projections per batch.

Device design notes:
  - The attention S^T = K^T.T @ Q^T matmuls have a K=64 contraction, so they
    are packed two-at-a-time into the 128x128 PE array via row tiling
    (tile_position (0,0)/(64,0) run CONCURRENTLY): heads 0 and 1 share one
    [q0;q1]/[k0;k1] tile pair and compute the same tk-block together; head 2
    uses duplicated [q2;q2]/[k2;k2] tiles and computes two consecutive
    tk-blocks together.
  - The kernel is ACT(exp)-bound: ~6.7M exp elements per core at 1 elem/
    lane/cycle @ 1.2 GHz is ~50us. The emission is SOFTWARE-PIPELINED so the
    exp stream never stalls on the in-order PE queue: for attention unit u,
    S(u) is emitted BEFORE [exp, mask, PV] of unit u-1, so S(u) always
    completes while exp(u-1) runs and exp(u) can start back-to-back.
    Production/projection fillers are emitted after each PV in small pieces.
  - V is augmented with a ones column per head: PV accumulation yields the
    softmax denominator as psum row 64 for free. Causal masking: matmul
    columns restricted to tq >= tk-block start; the diagonal 128x128
    sub-block is zeroed above the diagonal via affine_select (GpSimd) after
    exp (PV consumes pt afterwards; exp garbage in never-read columns is
    harmless).
  - Inputs arrive as a few large-row DMAs (6KB per partition for x chunks)
    instead of many 1KB-row transfers - DMA engines process ~2x faster.
  - A warm-up burst of matmuls at t=0 flips the PE HAM clock gate to 2.4GHz
    during the input-DMA window; a dummy exp preloads the ACT spline table.
  - Output is written in chunked [i, n, 128, 512] layout (contiguous DMA);
    host reassembles and reduces.
"""

import os
import sys

sys.path.insert(0, "/opt/trn_rl_repo")

import ml_dtypes
import numpy as np

import concourse.bass as bass
import concourse.tile as tile
from concourse import bacc, mybir
from concourse import bass_utils

B, T, C = 2, 2048, 768
H, D = 12, 64
N_CORES = 8
H_LOC = 3           # heads per core
DL = H_LOC * D      # 192 local head dims
TQ = 512            # tq chunk (psum bank width)
TB = 128            # tk block
NCH = T // TQ       # 4 chunks
NBL = TQ // TB      # 4 blocks per chunk
NKT = C // 128      # 6 contraction k-tiles
DV = D + 1          # head dim + denominator column
WV_W = H_LOC * DV   # 195 packed v columns
NWARM = 20          # HAM warm-up matmuls (N=256 each, ~4.3us dense cold)

f32 = mybir.dt.float32
bf16 = mybir.dt.bfloat16
EXP = mybir.ActivationFunctionType.Exp

LAST_RESULT = None  # test harness reads exec_time_ns from here


def _build_program(use_bias: bool):
    from contextlib import ExitStack

    nc = bacc.Bacc("TRN2", target_bir_lowering=False, debug=False,
                   num_devices=N_CORES)

    xt_d = nc.dram_tensor("xt", [NCH, 128, NKT * TQ], bf16, kind="ExternalInput").ap()
    xt1_d = nc.dram_tensor("xt1", [1, TQ], bf16, kind="ExternalInput").ap()
    wqk_d = nc.dram_tensor("wqk", [128, 7 * 2 * DL], bf16, kind="ExternalInput").ap()
    wv_d = nc.dram_tensor("wv", [128, 7 * WV_W], bf16, kind="ExternalInput").ap()
    wp_d = nc.dram_tensor("wp", [128, 2 * C], bf16, kind="ExternalInput").ap()
    out_d = nc.dram_tensor("outc", [NCH, 128, (C // 128) * TQ], bf16,
                           kind="ExternalOutput").ap()

    with tile.TileContext(nc) as tc, ExitStack() as ctx:
        cpool = ctx.enter_context(tc.tile_pool(name="const", bufs=1))
        wpool = ctx.enter_context(tc.tile_pool(name="w", bufs=1))
        xpool = ctx.enter_context(tc.tile_pool(name="x", bufs=1))
        qkpool = ctx.enter_context(tc.tile_pool(name="qk", bufs=1))

        # PSUM budget (8 banks): s 2x2 + po 2x1 + mix 2x1.
        s_ps = ctx.enter_context(tc.tile_pool(name="s_ps", bufs=2, space="PSUM"))
        po_ps = ctx.enter_context(tc.tile_pool(name="po_ps", bufs=2, space="PSUM"))
        mix_ps = ctx.enter_context(tc.tile_pool(name="mix_ps", bufs=2, space="PSUM"))
        pt_p = ctx.enter_context(tc.tile_pool(name="pt_p", bufs=10))
        nrm = ctx.enter_context(tc.tile_pool(name="nrm", bufs=4))
        outp = ctx.enter_context(tc.tile_pool(name="outp", bufs=5))

        ones_b = cpool.tile([1, D], bf16)
        nc.vector.memset(ones_b[:], 1.0)

        # --- HAM warm-up: keep the PE busy from t=0 so the clock gate flips
        # to 2.4 GHz during the input-DMA window, not 15us into compute.
        wmt = cpool.tile([128, TQ], bf16)
        nc.vector.memset(wmt[:], 0.0)
        for w in range(NWARM):
            wps = mix_ps.tile([128, TQ], f32, tag="mix", name=f"warm{w}")
            nc.tensor.matmul(wps[:, 0:256], wmt[:, 0:128], wmt[:, 0:256],
                             start=True, stop=True)
        # ACT spline-table preload (~2.7us) off the critical path.
        actw = nrm.tile([128, 1], f32, tag="actw", name="actw")
        nc.scalar.activation(actw[:], wmt[:, 0:1], EXP)

        # --- input loads: one large-row DMA per tensor / x chunk. x chunk 0
        # goes FIRST (it gates the first production), wqk in m-major slices
        # right after (production group m needs only its own slice).
        xt_c = []
        for t in range(NCH):
            t_ = xpool.tile([128, NKT * TQ], bf16, tag=f"xtc{t}", name=f"xtc{t}")
            xt_c.append(t_)
        hw = NKT * TQ // 2
        nc.sync.dma_start(xt_c[0][:, 0:hw], xt_d[0, :, 0:hw])
        nc.sync.dma_start(xt_c[0][:, hw:], xt_d[0, :, hw:])
        wqk_t = wpool.tile([128, 7 * 2 * DL], bf16)
        for m in range(3):
            nc.sync.dma_start(wqk_t[:, 896 * m : 896 * (m + 1)],
                              wqk_d[:, 896 * m : 896 * (m + 1)])

        def wqk_mj(m, j):
            return wqk_t[:, 896 * m + 128 * j : 896 * m + 128 * (j + 1)]

        wv_t = wpool.tile([128, 7 * WV_W], bf16)
        nc.sync.dma_start(wv_t[:], wv_d[:])
        wv = [wv_t[:, WV_W * j : WV_W * (j + 1)] for j in range(7)]
        for t in range(1, NCH):
            nc.sync.dma_start(xt_c[t][:], xt_d[t])
        if use_bias:
            xt1 = xpool.tile([1, TQ], bf16)
            nc.sync.dma_start(xt1[:], xt1_d[:])
        wp_t = wpool.tile([128, 2 * C], bf16)
        nc.sync.dma_start(wp_t[:], wp_d[:])
        wp = wp_t[:, 0:C]
        wp2 = wp_t[:, C : 2 * C]      # rows 64-127 are zeros (host pads)

        def xt(t, j):
            return xt_c[t][:, TQ * j : TQ * (j + 1)]

        # Attention-stage tiles (bf16). Heads 0/1 share [q0;q1]/[k0;k1]
        # tiles (the row-tiled S pair reads partition halves 0:64 / 64:128);
        # head 2 gets duplicated [q2;q2]/[k2;k2] so two consecutive
        # tk-blocks can run concurrently.
        qAB, kAB, q2d, k2d = [], [], [], []
        for t in range(NCH):
            qAB.append(qkpool.tile([128, TQ], bf16, tag=f"qAB{t}", name=f"qAB{t}"))
            kAB.append(qkpool.tile([128, TQ], bf16, tag=f"kAB{t}", name=f"kAB{t}"))
            q2d.append(qkpool.tile([128, TQ], bf16, tag=f"q2d{t}", name=f"q2d{t}"))
            k2d.append(qkpool.tile([128, TQ], bf16, tag=f"k2d{t}", name=f"k2d{t}"))
        v_sb = [qkpool.tile([128, WV_W], bf16, tag=f"v{t}", name=f"v{t}")
                for t in range(T // TB)]
        # per-chunk normalized-O^T tiles (per-tile deps: deferred proj of
        # chunk t-1 must not wait on chunk t's normalize)
        prhs0 = [qkpool.tile([128, TQ], bf16, tag=f"prhs0{t}", name=f"prhs0{t}")
                 for t in range(NCH)]      # heads 0,1
        prhs1 = [qkpool.tile([128, TQ], bf16, tag=f"prhs1{t}", name=f"prhs1{t}")
                 for t in range(NCH)]      # head 2, duplicated in both halves

        def emit_qk_group(t, m):
            # chunk t of q^T/k^T; M-tiles: [q0|q1], [k0|k1], [q2|k2]
            ps = mix_ps.tile([128, TQ], f32, tag="mix", name=f"ps_{t}_{m}")
            for j in range(NKT):
                nc.tensor.matmul(
                    ps[:],
                    wqk_mj(m, j),
                    xt(t, j),
                    start=(j == 0),
                    stop=(j == NKT - 1 and not use_bias),
                )
            if use_bias:
                nc.tensor.matmul(
                    ps[:], wqk_mj(m, 6)[0:1, :],
                    xt1[:], start=False, stop=True,
                )
            if m == 0:
                nc.vector.tensor_copy(qAB[t][:], ps[:])                # q0;q1
            elif m == 1:
                nc.vector.tensor_copy(kAB[t][:], ps[:])                # k0;k1
            else:
                nc.vector.tensor_copy(q2d[t][0:64, :], ps[0:64, :])    # q2
                nc.vector.tensor_copy(k2d[t][0:64, :], ps[64:128, :])  # k2
                # duplicate into the upper partition half (fast SBUF copy)
                nc.vector.tensor_copy(q2d[t][64:128, :], q2d[t][0:64, :])
                nc.vector.tensor_copy(k2d[t][64:128, :], k2d[t][0:64, :])

        def emit_v_group(t, tb):
            # v block tb in [t, d] layout; wv interleaves [v_h | ones] per
            # head. Without bias the ones columns are memset directly.
            psv = mix_ps.tile([128, TQ], f32, tag="mix", name=f"psv_{tb}")
            for j in range(NKT):
                nc.tensor.matmul(
                    psv[0:128, 0:WV_W],
                    xt_c[t][:, TQ * j + TB * (tb % NBL) : TQ * j + TB * (tb % NBL + 1)],
                    wv[j],
                    start=(j == 0), stop=(j == NKT - 1 and not use_bias),
                )
            if use_bias:
                nc.tensor.matmul(
                    psv[0:128, 0:WV_W],
                    xt1[0:1, 0:TB],
                    wv[6][0:1, :],
                    start=False, stop=True,
                )
            nc.vector.tensor_copy(v_sb[tb][:], psv[:, 0:WV_W])
            if not use_bias:
                for h in range(H_LOC):
                    c1 = DV * h + D
                    nc.gpsimd.memset(v_sb[tb][:, c1 : c1 + 1], 1.0)

        def emit_norm_pair(i, po0, po1):
            # row D of po is the softmax denominator. Broadcast both heads'
            # denominators into one [128, TQ] psum via two CONCURRENT
            # col-tiled rank-1 matmuls, one reciprocal, two multiplies.
            d0 = nrm.tile([1, TQ], bf16, tag="d", name=f"d0_{i}")
            d1 = nrm.tile([1, TQ], bf16, tag="d", name=f"d1_{i}")
            nc.vector.tensor_copy(d0[:], po0[D : D + 1, :])
            nc.vector.tensor_copy(d1[:], po1[D : D + 1, :])
            pb = mix_ps.tile([128, TQ], f32, tag="mix", name=f"pbp_{i}")
            nc.tensor.matmul(pb[0:D, :], ones_b[:], d0[:], start=True, stop=True)
            nc.tensor.matmul(pb[D : 2 * D, :], ones_b[:], d1[:],
                             start=True, stop=True)
            rb = nrm.tile([128, TQ], f32, tag="rb", name=f"rbp_{i}")
            nc.vector.reciprocal_approx_fast(rb[:], pb[:])
            nc.vector.tensor_mul(prhs0[i][0:D, :], po0[0:D, :], rb[0:D, :])
            nc.vector.tensor_mul(prhs0[i][D : 2 * D, :], po1[0:D, :],
                                 rb[D : 2 * D, :])

        def emit_norm_h2(i, po):
            d_sb = nrm.tile([1, TQ], bf16, tag="d", name=f"d2_{i}")
            nc.vector.tensor_copy(d_sb[:], po[D : D + 1, :])
            pb = mix_ps.tile([128, TQ], f32, tag="mix", name=f"pb2_{i}")
            nc.tensor.matmul(pb[0:D, :], ones_b[:], d_sb[:],
                             start=True, stop=True)
            rb = nrm.tile([D, TQ], f32, tag="rb2", name=f"rb2_{i}")
            nc.vector.reciprocal_approx_fast(rb[:], pb[0:D, :])
            nc.vector.tensor_mul(prhs1[i][0:64, :], po[0:D, :], rb[:])
            # duplicate h2 rows so the K=64 projection half can row-tile
            nc.vector.tensor_copy(prhs1[i][64:128, :], prhs1[i][0:64, :])

        osb_c = [outp.tile([128, (C // 128) * TQ], bf16, tag=f"osb{i}",
                           name=f"osb{i}", bufs=1) for i in range(NCH)]

        def emit_proj_pair(i, n):
            # projection chunks n, n+1. The K=128 prhs0 halves run as normal
            # full-array matmuls; the two K=64 prhs1 halves are packed into
            # the PE array concurrently via row tiling (wp2/prhs1 hold the
            # same data in both partition halves).
            ppa = mix_ps.tile([128, TQ], f32, tag="mix", name=f"ppa_{i}_{n}")
            ppb = mix_ps.tile([128, TQ], f32, tag="mix", name=f"ppb_{i}_{n}")
            nc.tensor.matmul(ppa[:], wp[:, 128 * n : 128 * (n + 1)],
                             prhs0[i][:], start=True, stop=False)
            nc.tensor.matmul(ppb[:], wp[:, 128 * (n + 1) : 128 * (n + 2)],
                             prhs0[i][:], start=True, stop=False)
            nc.tensor.matmul(ppa[:], wp2[0:64, 128 * n : 128 * (n + 1)],
                             prhs1[i][0:64, :], start=False, stop=True)
            nc.tensor.matmul(ppb[:], wp2[64:128, 128 * (n + 1) : 128 * (n + 2)],
                             prhs1[i][64:128, :], start=False, stop=True)
            for k, pp in ((0, ppa), (1, ppb)):
                dst = osb_c[i][:, TQ * (n + k) : TQ * (n + k + 1)]
                if i == NCH - 1 and k == 1:
                    # tail: ACT is done with exps - split copies across
                    # engines so the last chunk's output drains faster
                    nc.scalar.copy(dst, pp[:])
                else:
                    nc.vector.tensor_copy(dst, pp[:])
            if n + 2 == C // 128:
                # all six column chunks written -> one large-row DMA out
                nc.sync.dma_start(out_d[i], osb_c[i][:])

        # ------------------------------------------------------------------
        # Attention units, software-pipelined: iteration u emits S(u) on the
        # PE queue, THEN [exp, mask, PV] of unit u-1, then a filler piece.
        # S(u) therefore always runs during exp(u-1) and the ACT exp stream
        # never waits on PV/filler work queued behind it.
        # ------------------------------------------------------------------
        po_t = {}

        def emit_S(u):
            kind, i, p = u
            if kind == "pair":
                j = p - NBL * i
                c0 = 0 if j < 0 else TB * j
                ps2 = s_ps.tile([128, 2 * TQ], f32, tag="s", name=f"sp_{i}_{p}")
                blk = slice(TB * (p % NBL), TB * (p % NBL + 1))
                nc.tensor.matmul(ps2[:, c0:TQ],
                                 kAB[p // NBL][0:64, blk],
                                 qAB[i][0:64, c0:TQ], start=True, stop=True)
                nc.tensor.matmul(ps2[:, TQ + c0 : 2 * TQ],
                                 kAB[p // NBL][64:128, blk],
                                 qAB[i][64:128, c0:TQ], start=True, stop=True)
                return (ps2, (c0, c0))
            else:
                ps2 = s_ps.tile([128, 2 * TQ], f32, tag="s", name=f"s2_{i}_{p}")
                c0s = []
                for half in range(2):
                    Bq = 2 * p + half
                    j = Bq - NBL * i
                    c0 = 0 if j < 0 else TB * j
                    c0s.append(c0)
                    off = TQ * half
                    nc.tensor.matmul(
                        ps2[:, off + c0 : off + TQ],
                        k2d[Bq // NBL][64 * half : 64 * (half + 1),
                                       TB * (Bq % NBL) : TB * (Bq % NBL + 1)],
                        q2d[i][64 * half : 64 * (half + 1), c0:TQ],
                        start=True, stop=True,
                    )
                return (ps2, tuple(c0s))

        def emit_rest(u, ps2, c0s):
            kind, i, p = u
            nblk = NBL * (i + 1)
            pt = pt_p.tile([128, 2 * TQ], bf16, tag="pt", name=f"pt_{kind}_{i}_{p}")
            nc.scalar.activation(pt[:, c0s[0] :], ps2[:, c0s[0] :], EXP)
            if kind == "pair":
                if p == 0:
                    po_t[(i, 0)] = po_ps.tile([DV, TQ], f32, tag="po",
                                              name=f"po0_{i}")
                    po_t[(i, 1)] = po_ps.tile([DV, TQ], f32, tag="po",
                                              name=f"po1_{i}")
                j = p - NBL * i
                c0 = c0s[0]
                for half in range(2):
                    off = TQ * half
                    if j >= 0:
                        nc.gpsimd.affine_select(
                            pt[:, off + TB * j : off + TB * (j + 1)],
                            pt[:, off + TB * j : off + TB * (j + 1)],
                            pattern=[[1, TB]],
                            compare_op=mybir.AluOpType.is_ge,
                            fill=0.0,
                            base=0,
                            channel_multiplier=-1,
                        )
                    nc.tensor.matmul(
                        po_t[(i, half)][:, c0:TQ],
                        v_sb[p][:, DV * half : DV * (half + 1)],
                        pt[:, off + c0 : off + TQ],
                        start=(p == 0), stop=(p == nblk - 1),
                    )
                if p == nblk - 1:
                    emit_norm_pair(i, po_t[(i, 0)], po_t[(i, 1)])
            else:
                if p == 0:
                    po_t[(i, 2)] = po_ps.tile([DV, TQ], f32, tag="po",
                                              name=f"po2_{i}")
                for half in range(2):
                    Bq = 2 * p + half
                    j = Bq - NBL * i
                    c0 = c0s[half]
                    off = TQ * half
                    if j >= 0:
                        nc.gpsimd.affine_select(
                            pt[:, off + TB * j : off + TB * (j + 1)],
                            pt[:, off + TB * j : off + TB * (j + 1)],
                            pattern=[[1, TB]],
                            compare_op=mybir.AluOpType.is_ge,
                            fill=0.0,
                            base=0,
                            channel_multiplier=-1,
                        )
                    nc.tensor.matmul(
                        po_t[(i, 2)][:, c0:TQ],
                        v_sb[Bq][:, 2 * DV : 3 * DV],
                        pt[:, off + c0 : off + TQ],
                        start=(Bq == 0), stop=(Bq == nblk - 1),
                    )
                if p == nblk // 2 - 1:
                    emit_norm_h2(i, po_t[(i, 2)])

        def run_piece(piece):
            kind, a, b = piece
            if kind == "qk":
                emit_qk_group(a, b)
            elif kind == "v":
                emit_v_group(a, b)
            else:
                emit_proj_pair(a, b)

        # Filler pieces with DEADLINES (latest unit index at which the piece
        # must be emitted so its consumer's dependency order is correct) and
        # PE costs. Pieces pop when the accrued slack budget covers them, or
        # when forced by their deadline - this spreads PE filler work into
        # the exp-bound attention stream instead of bursting it.
        units = []
        for i in range(NCH):
            nblk = NBL * (i + 1)
            units += [("pair", i, p) for p in range(nblk)]
            units += [("h2", i, p) for p in range(nblk // 2)]
        first_u = {i: units.index(("pair", i, 0)) for i in range(NCH)}
        first_h2 = {i: units.index(("h2", i, 0)) for i in range(NCH)}

        fill_q = []   # (deadline, cost, piece); kept in emission order
        for t in range(NCH):
            if t > 0:
                fill_q += [(first_u[t] - 1, 1300, ("qk", t, m)) for m in (0, 1)]
                fill_q += [(first_h2[t] - 1, 1300, ("qk", t, 2))]
            fill_q += [(first_u[t] + tb - NBL * t, 550, ("v", t, tb))
                       for tb in range(NBL * t, NBL * (t + 1))]
        for i in range(NCH - 1):
            fill_q += [(10 ** 6, 750, ("proj", i, n))
                       for n in range(0, C // 128, 2)]
        fill_q.sort(key=lambda x: x[0])

        # chunk-0 q/k production before the pipeline starts
        for m in range(3):
            emit_qk_group(0, m)

        SLACK, BCAP = 450.0, 2500.0
        budget = 0.0
        pending = None
        for u_idx, u in enumerate(units):
            sctx = emit_S(u)
            if pending is not None:
                emit_rest(*pending)
            budget = min(budget + SLACK, BCAP)
            while fill_q:
                dl, cost, (kind, a, b) = fill_q[0]
                if kind == "proj" and u[1] <= a:
                    break
                if dl > u_idx and budget < cost:
                    break
                budget = max(0.0, budget - cost)
                run_piece(fill_q.pop(0)[2])
            pending = (u, *sctx)
        emit_rest(*pending)
        while fill_q:
            run_piece(fill_q.pop(0)[2])
        for n in range(0, C // 128, 2):
            emit_proj_pair(NCH - 1, n)

    nc.compile()
    return nc


_PROG_CACHE = {}


def kernel(x, Wqkv, bqkv, Wproj, bproj):
    global LAST_RESULT
    x = np.asarray(x, dtype=np.float32)
    Wqkv = np.asarray(Wqkv, dtype=np.float32)
    bqkv = np.asarray(bqkv, dtype=np.float32)
    Wproj = np.asarray(Wproj, dtype=np.float32)
    bproj = np.asarray(bproj, dtype=np.float32)

    Wq, Wk, Wv = Wqkv[:, 0:C], Wqkv[:, C : 2 * C], Wqkv[:, 2 * C : 3 * C]
    bq, bk, bv = bqkv[0:C], bqkv[C : 2 * C], bqkv[2 * C : 3 * C]
    scale = 1.0 / np.sqrt(D)

    use_bias = bool(np.any(bq) or np.any(bk) or np.any(bv))
    if use_bias not in _PROG_CACHE:
        _PROG_CACHE[use_bias] = _build_program(use_bias)
    nc = _PROG_CACHE[use_bias]

    in_maps = []
    for c in range(N_CORES):
        b = c // (N_CORES // B)
        g = c % (N_CORES // B)
        hs = slice(DL * g, DL * (g + 1))       # this core's head-dim rows/cols

        # x^T packed per (chunk, k-tile): [NCH, 128, NKT*TQ]
        xt = np.ascontiguousarray(
            x[b].T.reshape(NKT, 128, NCH, TQ).transpose(2, 1, 0, 3)
        ).reshape(NCH, 128, NKT * TQ)
        xt1 = np.ones((1, TQ), np.float32)

        wq_loc = Wq[:, hs] * scale             # fold 1/sqrt(D) into q
        bq_loc = bq[hs] * scale
        wk_loc, bk_loc = Wk[:, hs], bk[hs]
        wv_loc, bv_loc = Wv[:, hs], bv[hs]

        wqk = np.zeros((C + 128, 2 * DL), np.float32)   # 7 k-tiles of 128
        wqk[0:C, 0:128] = wq_loc[:, 0:128]
        wqk[C, 0:128] = bq_loc[0:128]
        wqk[0:C, 128:256] = wk_loc[:, 0:128]
        wqk[C, 128:256] = bk_loc[0:128]
        wqk[0:C, 256:320] = wq_loc[:, 128:192]
        wqk[C, 256:320] = bq_loc[128:192]
        wqk[0:C, 320:384] = wk_loc[:, 128:192]
        wqk[C, 320:384] = bk_loc[128:192]
        # m-major: [m, j, 128] per partition row (production group m only
        # needs its own contiguous 896-column slice)
        wqk = np.concatenate(
            [np.ascontiguousarray(
                wqk[:, 128 * m : 128 * (m + 1)].reshape(7, 128, 128)
                .transpose(1, 0, 2)).reshape(128, 896)
             for m in range(3)], axis=1)

        wv_pad = np.zeros((C + 128, WV_W), np.float32)
        for h in range(H_LOC):
            c0 = DV * h
            wv_pad[0:C, c0 : c0 + D] = wv_loc[:, D * h : D * (h + 1)]
            wv_pad[C, c0 : c0 + D] = bv_loc[D * h : D * (h + 1)]
            wv_pad[C, c0 + D] = 1.0            # ones column -> softmax denom
        wv_pad = np.ascontiguousarray(
            wv_pad.reshape(7, 128, WV_W).transpose(1, 0, 2)).reshape(128, -1)

        wp = np.zeros((2, 128, C), np.float32)
        wp[0] = Wproj[DL * g : DL * g + 128, :]  # cast to bf16 below
        wp[1, 0:64] = Wproj[DL * g + 128 : DL * (g + 1), :]
        wp[1, 64:128] = wp[1, 0:64]              # dup for row-tiled proj half
        wp = np.ascontiguousarray(wp.transpose(1, 0, 2)).reshape(128, 2 * C)

        bf = ml_dtypes.bfloat16
        in_maps.append({"xt": xt.astype(bf), "xt1": xt1.astype(bf),
                        "wqk": wqk.astype(bf), "wv": wv_pad.astype(bf),
                        "wp": wp.astype(bf)})

    res = bass_utils.run_bass_kernel_spmd(nc, in_maps, core_ids=list(range(N_CORES)))
    LAST_RESULT = res

    out = np.zeros((B, T, C), np.float32)
    for c in range(N_CORES):
        b = c // (N_CORES // B)
        # outc [i, 128, n*512] -> [C, T] -> [T, C]
        outT = (res.results[c]["outc"].astype(np.float32)
                .reshape(NCH, 128, C // 128, TQ)
                .transpose(2, 1, 0, 3).reshape(C, T))
        out[b] += outT.T
    return out + bproj


if __name__ == "__main__":
    rng = np.random.default_rng(0)
    s = 1.0 / np.sqrt(C)
    ins = {
        "x": rng.standard_normal((B, T, C), dtype=np.float32),
        "Wqkv": rng.standard_normal((C, 3 * C), dtype=np.float32) * s,
        "bqkv": np.zeros(3 * C, np.float32),
        "Wproj": rng.standard_normal((C, C), dtype=np.float32) * s,
        "bproj": np.zeros(C, np.float32),
    }
    out = kernel(**ins)
    print("out", out.shape, out.dtype, float(np.abs(out).max()))


# revision 29
# speedup vs baseline: 1.1239x; 1.1239x over previous
"""Multi-head causal self-attention (B=2, T=2048, C=768, H=12, D=64) on 8
Trainium2 NeuronCores.

Sharding: 24 (batch, head) units -> 3 heads per core; cores 0-3 take batch 0,
cores 4-7 take batch 1. Each core computes q/k/v projections for its 3 heads,
flash-style causal attention fully on-chip (no T x T tensor ever touches HBM),
and a partial output projection with its 192-row slice of Wproj. The host sums
the 4 partial projections per batch.

Device design notes:
  - The attention S^T = K^T.T @ Q^T matmuls have a K=64 contraction, so they
    are packed two-at-a-time into the 128x128 PE array via row tiling
    (tile_position (0,0)/(64,0) run CONCURRENTLY): heads 0 and 1 share one
    [q0;q1]/[k0;k1] tile pair and compute the same tk-block together; head 2
    uses duplicated [q2;q2]/[k2;k2] tiles and computes two consecutive
    tk-blocks together.
  - The kernel is ACT(exp)-bound: ~6.7M exp elements per core at 1 elem/
    lane/cycle @ 1.2 GHz is ~50us. The emission is SOFTWARE-PIPELINED so the
    exp stream never stalls on the in-order PE queue: for attention unit u,
    S(u) is emitted BEFORE [exp, mask, PV] of unit u-1, so S(u) always
    completes while exp(u-1) runs and exp(u) can start back-to-back.
    Production/projection fillers are emitted after each PV in small pieces.
  - V is augmented with a ones column per head: PV accumulation yields the
    softmax denominator as psum row 64 for free. Causal masking: matmul
    columns restricted to tq >= tk-block start; the diagonal 128x128
    sub-block is zeroed above the diagonal via affine_select (GpSimd) after
    exp (PV consumes pt afterwards; exp garbage in never-read columns is
    harmless).
  - Inputs arrive as a few large-row DMAs (6KB per partition for x chunks)
    instead of many 1KB-row transfers - DMA engines process ~2x faster.
  - A warm-up burst of matmuls at t=0 flips the PE HAM clock gate to 2.4GHz
    during the input-DMA window; a dummy exp preloads the ACT spline table.
  - Output is written in chunked [i, n, 128, 512] layout (contiguous DMA);
    host reassembles and reduces.
"""

import os
import sys

sys.path.insert(0, "/opt/trn_rl_repo")

import ml_dtypes
import numpy as np

import concourse.bass as bass
import concourse.tile as tile
from concourse import bacc, mybir
from concourse import bass_utils

B, T, C = 2, 2048, 768
H, D = 12, 64
N_CORES = 8
H_LOC = 3           # heads per core
DL = H_LOC * D      # 192 local head dims
TQ = 512            # tq chunk (psum bank width)
TB = 128            # tk block
NCH = T // TQ       # 4 chunks
NBL = TQ // TB      # 4 blocks per chunk
NKT = C // 128      # 6 contraction k-tiles
DV = D + 1          # head dim + denominator column
WV_W = H_LOC * DV   # 195 packed v columns
NWARM = 28          # HAM warm-up matmuls (N=256): stay dense until x0 lands

f32 = mybir.dt.float32
bf16 = mybir.dt.bfloat16
EXP = mybir.ActivationFunctionType.Exp

LAST_RESULT = None  # test harness reads exec_time_ns from here


def _build_program(use_bias: bool):
    from contextlib import ExitStack

    nc = bacc.Bacc("TRN2", target_bir_lowering=False, debug=False,
                   num_devices=N_CORES)

    xt_d = nc.dram_tensor("xt", [NCH, 128, NKT * TQ], bf16, kind="ExternalInput").ap()
    xt1_d = nc.dram_tensor("xt1", [1, TQ], bf16, kind="ExternalInput").ap()
    wqk_d = nc.dram_tensor("wqk", [128, 7 * 2 * DL], bf16, kind="ExternalInput").ap()
    wv_d = nc.dram_tensor("wv", [128, 7 * WV_W], bf16, kind="ExternalInput").ap()
    wp_d = nc.dram_tensor("wp", [128, 2 * C], bf16, kind="ExternalInput").ap()
    out_d = nc.dram_tensor("outc", [NCH, 128, (C // 128) * TQ], bf16,
                           kind="ExternalOutput").ap()

    with tile.TileContext(nc) as tc, ExitStack() as ctx:
        cpool = ctx.enter_context(tc.tile_pool(name="const", bufs=1))
        wpool = ctx.enter_context(tc.tile_pool(name="w", bufs=1))
        xpool = ctx.enter_context(tc.tile_pool(name="x", bufs=1))
        qkpool = ctx.enter_context(tc.tile_pool(name="qk", bufs=1))

        # PSUM budget (8 banks): s 2x2 + po 2x1 + mix 2x1.
        s_ps = ctx.enter_context(tc.tile_pool(name="s_ps", bufs=2, space="PSUM"))
        po_ps = ctx.enter_context(tc.tile_pool(name="po_ps", bufs=2, space="PSUM"))
        mix_ps = ctx.enter_context(tc.tile_pool(name="mix_ps", bufs=2, space="PSUM"))
        pt_p = ctx.enter_context(tc.tile_pool(name="pt_p", bufs=10))
        nrm = ctx.enter_context(tc.tile_pool(name="nrm", bufs=4))
        outp = ctx.enter_context(tc.tile_pool(name="outp", bufs=5))

        ones_b = cpool.tile([1, D], bf16)
        nc.vector.memset(ones_b[:], 1.0)

        # --- HAM warm-up: keep the PE busy from t=0 so the clock gate flips
        # to 2.4 GHz during the input-DMA window, not 15us into compute.
        wmt = cpool.tile([128, TQ], bf16)
        nc.vector.memset(wmt[:], 0.0)
        for w in range(NWARM):
            wps = mix_ps.tile([128, TQ], f32, tag="mix", name=f"warm{w}")
            nc.tensor.matmul(wps[:, 0:256], wmt[:, 0:128], wmt[:, 0:256],
                             start=True, stop=True)
        # ACT spline-table preload (~2.7us) off the critical path.
        actw = nrm.tile([128, 1], f32, tag="actw", name="actw")
        nc.scalar.activation(actw[:], wmt[:, 0:1], EXP)

        # --- input loads: one large-row DMA per tensor / x chunk. x chunk 0
        # goes FIRST (it gates the first production), wqk in m-major slices
        # right after (production group m needs only its own slice).
        xt_c = []
        for t in range(NCH):
            t_ = xpool.tile([128, NKT * TQ], bf16, tag=f"xtc{t}", name=f"xtc{t}")
            xt_c.append(t_)
        hw = NKT * TQ // 2
        nc.sync.dma_start(xt_c[0][:, 0:hw], xt_d[0, :, 0:hw])
        nc.sync.dma_start(xt_c[0][:, hw:], xt_d[0, :, hw:])
        wqk_t = wpool.tile([128, 7 * 2 * DL], bf16)
        for m in range(3):
            nc.sync.dma_start(wqk_t[:, 896 * m : 896 * (m + 1)],
                              wqk_d[:, 896 * m : 896 * (m + 1)])

        def wqk_mj(m, j):
            return wqk_t[:, 896 * m + 128 * j : 896 * m + 128 * (j + 1)]

        wv_t = wpool.tile([128, 7 * WV_W], bf16)
        nc.sync.dma_start(wv_t[:], wv_d[:])
        wv = [wv_t[:, WV_W * j : WV_W * (j + 1)] for j in range(7)]
        for t in range(1, NCH):
            nc.sync.dma_start(xt_c[t][:], xt_d[t])
        if use_bias:
            xt1 = xpool.tile([1, TQ], bf16)
            nc.sync.dma_start(xt1[:], xt1_d[:])
        wp_t = wpool.tile([128, 2 * C], bf16)
        nc.sync.dma_start(wp_t[:], wp_d[:])
        wp = wp_t[:, 0:C]
        wp2 = wp_t[:, C : 2 * C]      # rows 64-127 are zeros (host pads)

        def xt(t, j):
            return xt_c[t][:, TQ * j : TQ * (j + 1)]

        # Attention-stage tiles (bf16). Heads 0/1 share [q0;q1]/[k0;k1]
        # tiles (the row-tiled S pair reads partition halves 0:64 / 64:128);
        # head 2 gets duplicated [q2;q2]/[k2;k2] so two consecutive
        # tk-blocks can run concurrently.
        qAB, kAB, q2d, k2d = [], [], [], []
        for t in range(NCH):
            qAB.append(qkpool.tile([128, TQ], bf16, tag=f"qAB{t}", name=f"qAB{t}"))
            kAB.append(qkpool.tile([128, TQ], bf16, tag=f"kAB{t}", name=f"kAB{t}"))
            q2d.append(qkpool.tile([128, TQ], bf16, tag=f"q2d{t}", name=f"q2d{t}"))
            k2d.append(qkpool.tile([128, TQ], bf16, tag=f"k2d{t}", name=f"k2d{t}"))
        v_sb = [qkpool.tile([128, WV_W], bf16, tag=f"v{t}", name=f"v{t}")
                for t in range(T // TB)]
        # per-chunk normalized-O^T tiles (per-tile deps: deferred proj of
        # chunk t-1 must not wait on chunk t's normalize)
        prhs0 = [qkpool.tile([128, TQ], bf16, tag=f"prhs0{t}", name=f"prhs0{t}")
                 for t in range(NCH)]      # heads 0,1
        prhs1 = [qkpool.tile([128, TQ], bf16, tag=f"prhs1{t}", name=f"prhs1{t}")
                 for t in range(NCH)]      # head 2, duplicated in both halves

        def emit_qk_group(t, m):
            # chunk t of q^T/k^T; M-tiles: [q0|q1], [k0|k1], [q2|k2]
            ps = mix_ps.tile([128, TQ], f32, tag="mix", name=f"ps_{t}_{m}")
            for j in range(NKT):
                nc.tensor.matmul(
                    ps[:],
                    wqk_mj(m, j),
                    xt(t, j),
                    start=(j == 0),
                    stop=(j == NKT - 1 and not use_bias),
                )
            if use_bias:
                nc.tensor.matmul(
                    ps[:], wqk_mj(m, 6)[0:1, :],
                    xt1[:], start=False, stop=True,
                )
            if m == 0:
                nc.vector.tensor_copy(qAB[t][:], ps[:])                # q0;q1
            elif m == 1:
                nc.vector.tensor_copy(kAB[t][:], ps[:])                # k0;k1
            else:
                nc.vector.tensor_copy(q2d[t][0:64, :], ps[0:64, :])    # q2
                nc.vector.tensor_copy(k2d[t][0:64, :], ps[64:128, :])  # k2
                # duplicate into the upper partition half (fast SBUF copy)
                nc.vector.tensor_copy(q2d[t][64:128, :], q2d[t][0:64, :])
                nc.vector.tensor_copy(k2d[t][64:128, :], k2d[t][0:64, :])

        def emit_v_group(t, tb):
            # v block tb in [t, d] layout; wv interleaves [v_h | ones] per
            # head. Without bias the ones columns are memset directly.
            psv = mix_ps.tile([128, TQ], f32, tag="mix", name=f"psv_{tb}")
            for j in range(NKT):
                nc.tensor.matmul(
                    psv[0:128, 0:WV_W],
                    xt_c[t][:, TQ * j + TB * (tb % NBL) : TQ * j + TB * (tb % NBL + 1)],
                    wv[j],
                    start=(j == 0), stop=(j == NKT - 1 and not use_bias),
                )
            if use_bias:
                nc.tensor.matmul(
                    psv[0:128, 0:WV_W],
                    xt1[0:1, 0:TB],
                    wv[6][0:1, :],
                    start=False, stop=True,
                )
            nc.vector.tensor_copy(v_sb[tb][:], psv[:, 0:WV_W])
            if not use_bias:
                for h in range(H_LOC):
                    c1 = DV * h + D
                    nc.gpsimd.memset(v_sb[tb][:, c1 : c1 + 1], 1.0)

        def emit_norm_pair(i, po0, po1):
            # row D of po is the softmax denominator. Broadcast both heads'
            # denominators into one [128, TQ] psum via two CONCURRENT
            # col-tiled rank-1 matmuls, one reciprocal, two multiplies.
            d0 = nrm.tile([1, TQ], bf16, tag="d", name=f"d0_{i}")
            d1 = nrm.tile([1, TQ], bf16, tag="d", name=f"d1_{i}")
            nc.vector.tensor_copy(d0[:], po0[D : D + 1, :])
            nc.vector.tensor_copy(d1[:], po1[D : D + 1, :])
            pb = mix_ps.tile([128, TQ], f32, tag="mix", name=f"pbp_{i}")
            nc.tensor.matmul(pb[0:D, :], ones_b[:], d0[:], start=True, stop=True)
            nc.tensor.matmul(pb[D : 2 * D, :], ones_b[:], d1[:],
                             start=True, stop=True)
            rb = nrm.tile([128, TQ], f32, tag="rb", name=f"rbp_{i}")
            nc.vector.reciprocal_approx_fast(rb[:], pb[:])
            nc.vector.tensor_mul(prhs0[i][0:D, :], po0[0:D, :], rb[0:D, :])
            nc.vector.tensor_mul(prhs0[i][D : 2 * D, :], po1[0:D, :],
                                 rb[D : 2 * D, :])

        def emit_norm_h2(i, po):
            d_sb = nrm.tile([1, TQ], bf16, tag="d", name=f"d2_{i}")
            nc.vector.tensor_copy(d_sb[:], po[D : D + 1, :])
            pb = mix_ps.tile([128, TQ], f32, tag="mix", name=f"pb2_{i}")
            nc.tensor.matmul(pb[0:D, :], ones_b[:], d_sb[:],
                             start=True, stop=True)
            rb = nrm.tile([D, TQ], f32, tag="rb2", name=f"rb2_{i}")
            nc.vector.reciprocal_approx_fast(rb[:], pb[0:D, :])
            nc.vector.tensor_mul(prhs1[i][0:64, :], po[0:D, :], rb[:])
            # duplicate h2 rows so the K=64 projection half can row-tile
            nc.vector.tensor_copy(prhs1[i][64:128, :], prhs1[i][0:64, :])

        osb_c = [outp.tile([128, (C // 128) * TQ], bf16, tag=f"osb{i}",
                           name=f"osb{i}", bufs=1) for i in range(NCH)]

        def emit_proj_pair(i, n):
            # projection chunks n, n+1. The K=128 prhs0 halves run as normal
            # full-array matmuls; the two K=64 prhs1 halves are packed into
            # the PE array concurrently via row tiling (wp2/prhs1 hold the
            # same data in both partition halves).
            ppa = mix_ps.tile([128, TQ], f32, tag="mix", name=f"ppa_{i}_{n}")
            ppb = mix_ps.tile([128, TQ], f32, tag="mix", name=f"ppb_{i}_{n}")
            nc.tensor.matmul(ppa[:], wp[:, 128 * n : 128 * (n + 1)],
                             prhs0[i][:], start=True, stop=False)
            nc.tensor.matmul(ppb[:], wp[:, 128 * (n + 1) : 128 * (n + 2)],
                             prhs0[i][:], start=True, stop=False)
            nc.tensor.matmul(ppa[:], wp2[0:64, 128 * n : 128 * (n + 1)],
                             prhs1[i][0:64, :], start=False, stop=True)
            nc.tensor.matmul(ppb[:], wp2[64:128, 128 * (n + 1) : 128 * (n + 2)],
                             prhs1[i][64:128, :], start=False, stop=True)
            for k, pp in ((0, ppa), (1, ppb)):
                dst = osb_c[i][:, TQ * (n + k) : TQ * (n + k + 1)]
                if i == NCH - 1 and k == 1:
                    # tail: ACT is done with exps - split copies across
                    # engines so the last chunk's output drains faster
                    nc.scalar.copy(dst, pp[:])
                else:
                    nc.vector.tensor_copy(dst, pp[:])
            if n + 2 == C // 128:
                # all six column chunks written -> one large-row DMA out
                nc.sync.dma_start(out_d[i], osb_c[i][:])

        # ------------------------------------------------------------------
        # Attention units, software-pipelined: iteration u emits S(u) on the
        # PE queue, THEN [exp, mask, PV] of unit u-1, then a filler piece.
        # S(u) therefore always runs during exp(u-1) and the ACT exp stream
        # never waits on PV/filler work queued behind it.
        # ------------------------------------------------------------------
        po_t = {}

        def emit_S(u):
            kind, i, p = u
            if kind == "pair":
                j = p - NBL * i
                c0 = 0 if j < 0 else TB * j
                ps2 = s_ps.tile([128, 2 * TQ], f32, tag="s", name=f"sp_{i}_{p}")
                blk = slice(TB * (p % NBL), TB * (p % NBL + 1))
                nc.tensor.matmul(ps2[:, c0:TQ],
                                 kAB[p // NBL][0:64, blk],
                                 qAB[i][0:64, c0:TQ], start=True, stop=True)
                nc.tensor.matmul(ps2[:, TQ + c0 : 2 * TQ],
                                 kAB[p // NBL][64:128, blk],
                                 qAB[i][64:128, c0:TQ], start=True, stop=True)
                return (ps2, (c0, c0))
            else:
                ps2 = s_ps.tile([128, 2 * TQ], f32, tag="s", name=f"s2_{i}_{p}")
                c0s = []
                for half in range(2):
                    Bq = 2 * p + half
                    j = Bq - NBL * i
                    c0 = 0 if j < 0 else TB * j
                    c0s.append(c0)
                    off = TQ * half
                    nc.tensor.matmul(
                        ps2[:, off + c0 : off + TQ],
                        k2d[Bq // NBL][64 * half : 64 * (half + 1),
                                       TB * (Bq % NBL) : TB * (Bq % NBL + 1)],
                        q2d[i][64 * half : 64 * (half + 1), c0:TQ],
                        start=True, stop=True,
                    )
                return (ps2, tuple(c0s))

        def emit_rest(u, ps2, c0s):
            kind, i, p = u
            nblk = NBL * (i + 1)
            pt = pt_p.tile([128, 2 * TQ], bf16, tag="pt", name=f"pt_{kind}_{i}_{p}")
            nc.scalar.activation(pt[:, c0s[0] :], ps2[:, c0s[0] :], EXP)
            if kind == "pair":
                if p == 0:
                    po_t[(i, 0)] = po_ps.tile([DV, TQ], f32, tag="po",
                                              name=f"po0_{i}")
                    po_t[(i, 1)] = po_ps.tile([DV, TQ], f32, tag="po",
                                              name=f"po1_{i}")
                j = p - NBL * i
                c0 = c0s[0]
                for half in range(2):
                    off = TQ * half
                    if j >= 0:
                        nc.gpsimd.affine_select(
                            pt[:, off + TB * j : off + TB * (j + 1)],
                            pt[:, off + TB * j : off + TB * (j + 1)],
                            pattern=[[1, TB]],
                            compare_op=mybir.AluOpType.is_ge,
                            fill=0.0,
                            base=0,
                            channel_multiplier=-1,
                        )
                    nc.tensor.matmul(
                        po_t[(i, half)][:, c0:TQ],
                        v_sb[p][:, DV * half : DV * (half + 1)],
                        pt[:, off + c0 : off + TQ],
                        start=(p == 0), stop=(p == nblk - 1),
                    )
                if p == nblk - 1:
                    emit_norm_pair(i, po_t[(i, 0)], po_t[(i, 1)])
            else:
                if p == 0:
                    po_t[(i, 2)] = po_ps.tile([DV, TQ], f32, tag="po",
                                              name=f"po2_{i}")
                for half in range(2):
                    Bq = 2 * p + half
                    j = Bq - NBL * i
                    c0 = c0s[half]
                    off = TQ * half
                    if j >= 0:
                        nc.gpsimd.affine_select(
                            pt[:, off + TB * j : off + TB * (j + 1)],
                            pt[:, off + TB * j : off + TB * (j + 1)],
                            pattern=[[1, TB]],
                            compare_op=mybir.AluOpType.is_ge,
                            fill=0.0,
                            base=0,
                            channel_multiplier=-1,
                        )
                    nc.tensor.matmul(
                        po_t[(i, 2)][:, c0:TQ],
                        v_sb[Bq][:, 2 * DV : 3 * DV],
                        pt[:, off + c0 : off + TQ],
                        start=(Bq == 0), stop=(Bq == nblk - 1),
                    )
                if p == nblk // 2 - 1:
                    emit_norm_h2(i, po_t[(i, 2)])

        def run_piece(piece):
            kind, a, b = piece
            if kind == "qk":
                emit_qk_group(a, b)
            elif kind == "v":
                emit_v_group(a, b)
            else:
                emit_proj_pair(a, b)

        # Filler pieces with DEADLINES (latest unit index at which the piece
        # must be emitted so its consumer's dependency order is correct) and
        # PE costs. Pieces pop when the accrued slack budget covers them, or
        # when forced by their deadline - this spreads PE filler work into
        # the exp-bound attention stream instead of bursting it.
        units = []
        for i in range(NCH):
            nblk = NBL * (i + 1)
            units += [("pair", i, p) for p in range(nblk)]
            units += [("h2", i, p) for p in range(nblk // 2)]
        first_u = {i: units.index(("pair", i, 0)) for i in range(NCH)}
        first_h2 = {i: units.index(("h2", i, 0)) for i in range(NCH)}

        def vdl(t, tb):
            return first_u[t] + tb - NBL * t

        plan = {
            0: [(vdl(0, tb), 550, ("v", 0, tb)) for tb in range(NBL)] +
               [(first_u[1] - 1, 1300, ("qk", 1, 0)),
                (first_u[1] - 1, 1300, ("qk", 1, 1)),
                (first_h2[1] - 1, 1300, ("qk", 1, 2))],
            1: [(vdl(1, tb), 550, ("v", 1, tb)) for tb in range(NBL, 2 * NBL)] +
               [(first_u[2] - 1, 1300, ("qk", 2, 0)),
                (first_u[2] - 1, 1300, ("qk", 2, 1)),
                (first_h2[2] - 1, 1300, ("qk", 2, 2))] +
               [(vdl(2, tb), 550, ("v", 2, tb))
                for tb in range(2 * NBL, 3 * NBL)],
            2: [(vdl(3, tb), 550, ("v", 3, tb))
                for tb in range(3 * NBL, 4 * NBL)] +
               [(first_u[3] - 1, 1300, ("qk", 3, 0)),
                (first_u[3] - 1, 1300, ("qk", 3, 1)),
                (first_h2[3] - 1, 1300, ("qk", 3, 2))],
            3: [(10 ** 6, 750, ("proj", i, n)) for i in range(NCH - 1)
                for n in range(0, C // 128, 2)],
        }
        nu = {i: sum(1 for u in units if u[1] == i) for i in range(NCH)}
        rate = {i: sum(c for _, c, _ in plan[i]) / nu[i] for i in range(NCH)}

        # chunk-0 q/k production before the pipeline starts
        for m in range(3):
            emit_qk_group(0, m)

        budget = 0.0
        pending = None
        for u_idx, u in enumerate(units):
            sctx = emit_S(u)
            if pending is not None:
                emit_rest(*pending)
            i = u[1]
            budget += rate[i] + 1.0
            for t in range(i + 1):          # earlier chunks' leftovers first
                q = plan[t]
                while q:
                    dl, cost, (kind, a, b) = q[0]
                    if kind == "proj" and i <= a:
                        break
                    if dl > u_idx and budget < cost:
                        break
                    budget = max(0.0, budget - cost)
                    run_piece(q.pop(0)[2])
            pending = (u, *sctx)
        emit_rest(*pending)
        for i in range(NCH):
            while plan[i]:
                run_piece(plan[i].pop(0)[2])
        for n in range(0, C // 128, 2):
            emit_proj_pair(NCH - 1, n)

    nc.compile()
    return nc


_PROG_CACHE = {}


def kernel(x, Wqkv, bqkv, Wproj, bproj):
    global LAST_RESULT
    x = np.asarray(x, dtype=np.float32)
    Wqkv = np.asarray(Wqkv, dtype=np.float32)
    bqkv = np.asarray(bqkv, dtype=np.float32)
    Wproj = np.asarray(Wproj, dtype=np.float32)
    bproj = np.asarray(bproj, dtype=np.float32)

    Wq, Wk, Wv = Wqkv[:, 0:C], Wqkv[:, C : 2 * C], Wqkv[:, 2 * C : 3 * C]
    bq, bk, bv = bqkv[0:C], bqkv[C : 2 * C], bqkv[2 * C : 3 * C]
    scale = 1.0 / np.sqrt(D)

    use_bias = bool(np.any(bq) or np.any(bk) or np.any(bv))
    if use_bias not in _PROG_CACHE:
        _PROG_CACHE[use_bias] = _build_program(use_bias)
    nc = _PROG_CACHE[use_bias]

    in_maps = []
    for c in range(N_CORES):
        b = c // (N_CORES // B)
        g = c % (N_CORES // B)
        hs = slice(DL * g, DL * (g + 1))       # this core's head-dim rows/cols

        # x^T packed per (chunk, k-tile): [NCH, 128, NKT*TQ]
        xt = np.ascontiguousarray(
            x[b].T.reshape(NKT, 128, NCH, TQ).transpose(2, 1, 0, 3)
        ).reshape(NCH, 128, NKT * TQ)
        xt1 = np.ones((1, TQ), np.float32)

        wq_loc = Wq[:, hs] * scale             # fold 1/sqrt(D) into q
        bq_loc = bq[hs] * scale
        wk_loc, bk_loc = Wk[:, hs], bk[hs]
        wv_loc, bv_loc = Wv[:, hs], bv[hs]

        wqk = np.zeros((C + 128, 2 * DL), np.float32)   # 7 k-tiles of 128
        wqk[0:C, 0:128] = wq_loc[:, 0:128]
        wqk[C, 0:128] = bq_loc[0:128]
        wqk[0:C, 128:256] = wk_loc[:, 0:128]
        wqk[C, 128:256] = bk_loc[0:128]
        wqk[0:C, 256:320] = wq_loc[:, 128:192]
        wqk[C, 256:320] = bq_loc[128:192]
        wqk[0:C, 320:384] = wk_loc[:, 128:192]
        wqk[C, 320:384] = bk_loc[128:192]
        # m-major: [m, j, 128] per partition row (production group m only
        # needs its own contiguous 896-column slice)
        wqk = np.concatenate(
            [np.ascontiguousarray(
                wqk[:, 128 * m : 128 * (m + 1)].reshape(7, 128, 128)
                .transpose(1, 0, 2)).reshape(128, 896)
             for m in range(3)], axis=1)

        wv_pad = np.zeros((C + 128, WV_W), np.float32)
        for h in range(H_LOC):
            c0 = DV * h
            wv_pad[0:C, c0 : c0 + D] = wv_loc[:, D * h : D * (h + 1)]
            wv_pad[C, c0 : c0 + D] = bv_loc[D * h : D * (h + 1)]
            wv_pad[C, c0 + D] = 1.0            # ones column -> softmax denom
        wv_pad = np.ascontiguousarray(
            wv_pad.reshape(7, 128, WV_W).transpose(1, 0, 2)).reshape(128, -1)

        wp = np.zeros((2, 128, C), np.float32)
        wp[0] = Wproj[DL * g : DL * g + 128, :]  # cast to bf16 below
        wp[1, 0:64] = Wproj[DL * g + 128 : DL * (g + 1), :]
        wp[1, 64:128] = wp[1, 0:64]              # dup for row-tiled proj half
        wp = np.ascontiguousarray(wp.transpose(1, 0, 2)).reshape(128, 2 * C)

        bf = ml_dtypes.bfloat16
        in_maps.append({"xt": xt.astype(bf), "xt1": xt1.astype(bf),
                        "wqk": wqk.astype(bf), "wv": wv_pad.astype(bf),
                        "wp": wp.astype(bf)})

    res = bass_utils.run_bass_kernel_spmd(nc, in_maps, core_ids=list(range(N_CORES)))
    LAST_RESULT = res

    out = np.zeros((B, T, C), np.float32)
    for c in range(N_CORES):
        b = c // (N_CORES // B)
        # outc [i, 128, n*512] -> [C, T] -> [T, C]
        outT = (res.results[c]["outc"].astype(np.float32)
                .reshape(NCH, 128, C // 128, TQ)
                .transpose(2, 1, 0, 3).reshape(C, T))
        out[b] += outT.T
    return out + bproj


if __name__ == "__main__":
    rng = np.random.default_rng(0)
    s = 1.0 / np.sqrt(C)
    ins = {
        "x": rng.standard_normal((B, T, C), dtype=np.float32),
        "Wqkv": rng.standard_normal((C, 3 * C), dtype=np.float32) * s,
        "bqkv": np.zeros(3 * C, np.float32),
        "Wproj": rng.standard_normal((C, C), dtype=np.float32) * s,
        "bproj": np.zeros(C, np.float32),
    }
    out = kernel(**ins)
    print("out", out.shape, out.dtype, float(np.abs(out).max()))


# revision 31
# speedup vs baseline: 1.1259x; 1.0018x over previous
"""Multi-head causal self-attention (B=2, T=2048, C=768, H=12, D=64) on 8
Trainium2 NeuronCores.

Sharding: 24 (batch, head) units -> 3 heads per core; cores 0-3 take batch 0,
cores 4-7 take batch 1. Each core computes q/k/v projections for its 3 heads,
flash-style causal attention fully on-chip (no T x T tensor ever touches HBM),
and a partial output projection with its 192-row slice of Wproj. The host sums
the 4 partial projections per batch.

Device design notes:
  - The attention S^T = K^T.T @ Q^T matmuls have a K=64 contraction, so they
    are packed two-at-a-time into the 128x128 PE array via row tiling
    (tile_position (0,0)/(64,0) run CONCURRENTLY): heads 0 and 1 share one
    [q0;q1]/[k0;k1] tile pair and compute the same tk-block together; head 2
    uses duplicated [q2;q2]/[k2;k2] tiles and computes two consecutive
    tk-blocks together.
  - The kernel is ACT(exp)-bound: ~6.7M exp elements per core at 1 elem/
    lane/cycle @ 1.2 GHz is ~50us. The emission is SOFTWARE-PIPELINED so the
    exp stream never stalls on the in-order PE queue: for attention unit u,
    S(u) is emitted BEFORE [exp, mask, PV] of unit u-1, so S(u) always
    completes while exp(u-1) runs and exp(u) can start back-to-back.
    Production/projection fillers are emitted after each PV in small pieces.
  - V is augmented with a ones column per head: PV accumulation yields the
    softmax denominator as psum row 64 for free. Causal masking: matmul
    columns restricted to tq >= tk-block start; the diagonal 128x128
    sub-block is zeroed above the diagonal via affine_select (GpSimd) after
    exp (PV consumes pt afterwards; exp garbage in never-read columns is
    harmless).
  - Inputs arrive as a few large-row DMAs (6KB per partition for x chunks)
    instead of many 1KB-row transfers - DMA engines process ~2x faster.
  - A warm-up burst of matmuls at t=0 flips the PE HAM clock gate to 2.4GHz
    during the input-DMA window; a dummy exp preloads the ACT spline table.
  - Output is written in chunked [i, n, 128, 512] layout (contiguous DMA);
    host reassembles and reduces.
"""

import os
import sys

sys.path.insert(0, "/opt/trn_rl_repo")

import ml_dtypes
import numpy as np

import concourse.bass as bass
import concourse.tile as tile
from concourse import bacc, mybir
from concourse import bass_utils

B, T, C = 2, 2048, 768
H, D = 12, 64
N_CORES = 8
H_LOC = 3           # heads per core
DL = H_LOC * D      # 192 local head dims
TQ = 512            # tq chunk (psum bank width)
TB = 128            # tk block
NCH = T // TQ       # 4 chunks
NBL = TQ // TB      # 4 blocks per chunk
NKT = C // 128      # 6 contraction k-tiles
DV = D + 1          # head dim + denominator column
WV_W = H_LOC * DV   # 195 packed v columns
NWARM = 28          # HAM warm-up matmuls (N=256): stay dense until x0 lands

f32 = mybir.dt.float32
bf16 = mybir.dt.bfloat16
EXP = mybir.ActivationFunctionType.Exp

LAST_RESULT = None  # test harness reads exec_time_ns from here


def _build_program(use_bias: bool):
    from contextlib import ExitStack

    nc = bacc.Bacc("TRN2", target_bir_lowering=False, debug=False,
                   num_devices=N_CORES)

    xt_d = nc.dram_tensor("xt", [NCH, 128, NKT * TQ], bf16, kind="ExternalInput").ap()
    xt1_d = nc.dram_tensor("xt1", [1, TQ], bf16, kind="ExternalInput").ap()
    wqk_d = nc.dram_tensor("wqk", [128, 7 * 2 * DL], bf16, kind="ExternalInput").ap()
    wv_d = nc.dram_tensor("wv", [128, 7 * WV_W], bf16, kind="ExternalInput").ap()
    wp_d = nc.dram_tensor("wp", [128, 2 * C], bf16, kind="ExternalInput").ap()
    out_d = nc.dram_tensor("outc", [NCH, 128, (C // 128) * TQ], bf16,
                           kind="ExternalOutput").ap()

    with tile.TileContext(nc) as tc, ExitStack() as ctx:
        cpool = ctx.enter_context(tc.tile_pool(name="const", bufs=1))
        wpool = ctx.enter_context(tc.tile_pool(name="w", bufs=1))
        xpool = ctx.enter_context(tc.tile_pool(name="x", bufs=1))
        qkpool = ctx.enter_context(tc.tile_pool(name="qk", bufs=1))

        # PSUM budget (8 banks): s 2x2 + po 2x1 + mix 2x1.
        s_ps = ctx.enter_context(tc.tile_pool(name="s_ps", bufs=2, space="PSUM"))
        po_ps = ctx.enter_context(tc.tile_pool(name="po_ps", bufs=2, space="PSUM"))
        mix_ps = ctx.enter_context(tc.tile_pool(name="mix_ps", bufs=2, space="PSUM"))
        pt_p = ctx.enter_context(tc.tile_pool(name="pt_p", bufs=10))
        nrm = ctx.enter_context(tc.tile_pool(name="nrm", bufs=4))
        outp = ctx.enter_context(tc.tile_pool(name="outp", bufs=5))

        ones_b = cpool.tile([1, D], bf16)
        nc.vector.memset(ones_b[:], 1.0)

        # --- HAM warm-up: keep the PE busy from t=0 so the clock gate flips
        # to 2.4 GHz during the input-DMA window, not 15us into compute.
        wmt = cpool.tile([128, TQ], bf16)
        nc.vector.memset(wmt[:], 0.0)
        for w in range(NWARM):
            wps = mix_ps.tile([128, TQ], f32, tag="mix", name=f"warm{w}")
            nc.tensor.matmul(wps[:, 0:256], wmt[:, 0:128], wmt[:, 0:256],
                             start=True, stop=True)
        # ACT spline-table preload (~2.7us) off the critical path.
        actw = nrm.tile([128, 1], f32, tag="actw", name="actw")
        nc.scalar.activation(actw[:], wmt[:, 0:1], EXP)

        # --- input loads: one large-row DMA per tensor / x chunk. x chunk 0
        # goes FIRST (it gates the first production), wqk in m-major slices
        # right after (production group m needs only its own slice).
        xt_c = []
        for t in range(NCH):
            t_ = xpool.tile([128, NKT * TQ], bf16, tag=f"xtc{t}", name=f"xtc{t}")
            xt_c.append(t_)
        hw = NKT * TQ // 2
        nc.sync.dma_start(xt_c[0][:, 0:hw], xt_d[0, :, 0:hw])
        nc.sync.dma_start(xt_c[0][:, hw:], xt_d[0, :, hw:])
        wqk_t = wpool.tile([128, 7 * 2 * DL], bf16)
        for m in range(3):
            nc.sync.dma_start(wqk_t[:, 896 * m : 896 * (m + 1)],
                              wqk_d[:, 896 * m : 896 * (m + 1)])

        def wqk_mj(m, j):
            return wqk_t[:, 896 * m + 128 * j : 896 * m + 128 * (j + 1)]

        wv_t = wpool.tile([128, 7 * WV_W], bf16)
        nc.sync.dma_start(wv_t[:], wv_d[:])
        wv = [wv_t[:, WV_W * j : WV_W * (j + 1)] for j in range(7)]
        for t in range(1, NCH):
            nc.sync.dma_start(xt_c[t][:], xt_d[t])
        if use_bias:
            xt1 = xpool.tile([1, TQ], bf16)
            nc.sync.dma_start(xt1[:], xt1_d[:])
        wp_t = wpool.tile([128, 2 * C], bf16)
        nc.sync.dma_start(wp_t[:], wp_d[:])
        wp = wp_t[:, 0:C]
        wp2 = wp_t[:, C : 2 * C]      # rows 64-127 are zeros (host pads)

        def xt(t, j):
            return xt_c[t][:, TQ * j : TQ * (j + 1)]

        # Attention-stage tiles (bf16). Heads 0/1 share [q0;q1]/[k0;k1]
        # tiles (the row-tiled S pair reads partition halves 0:64 / 64:128);
        # head 2 gets duplicated [q2;q2]/[k2;k2] so two consecutive
        # tk-blocks can run concurrently.
        qAB, kAB, q2d, k2d = [], [], [], []
        for t in range(NCH):
            qAB.append(qkpool.tile([128, TQ], bf16, tag=f"qAB{t}", name=f"qAB{t}"))
            kAB.append(qkpool.tile([128, TQ], bf16, tag=f"kAB{t}", name=f"kAB{t}"))
            q2d.append(qkpool.tile([128, TQ], bf16, tag=f"q2d{t}", name=f"q2d{t}"))
            k2d.append(qkpool.tile([128, TQ], bf16, tag=f"k2d{t}", name=f"k2d{t}"))
        v_sb = [qkpool.tile([128, WV_W], bf16, tag=f"v{t}", name=f"v{t}")
                for t in range(T // TB)]
        # per-chunk normalized-O^T tiles (per-tile deps: deferred proj of
        # chunk t-1 must not wait on chunk t's normalize)
        prhs0 = [qkpool.tile([128, TQ], bf16, tag=f"prhs0{t}", name=f"prhs0{t}")
                 for t in range(NCH)]      # heads 0,1
        prhs1 = [qkpool.tile([128, TQ], bf16, tag=f"prhs1{t}", name=f"prhs1{t}")
                 for t in range(NCH)]      # head 2, duplicated in both halves

        def emit_qk_group(t, m):
            # chunk t of q^T/k^T; M-tiles: [q0|q1], [k0|k1], [q2|k2]
            ps = mix_ps.tile([128, TQ], f32, tag="mix", name=f"ps_{t}_{m}")
            for j in range(NKT):
                nc.tensor.matmul(
                    ps[:],
                    wqk_mj(m, j),
                    xt(t, j),
                    start=(j == 0),
                    stop=(j == NKT - 1 and not use_bias),
                )
            if use_bias:
                nc.tensor.matmul(
                    ps[:], wqk_mj(m, 6)[0:1, :],
                    xt1[:], start=False, stop=True,
                )
            if m == 0:
                nc.vector.tensor_copy(qAB[t][:], ps[:])                # q0;q1
            elif m == 1:
                nc.vector.tensor_copy(kAB[t][:], ps[:])                # k0;k1
            else:
                nc.vector.tensor_copy(q2d[t][0:64, :], ps[0:64, :])    # q2
                nc.vector.tensor_copy(k2d[t][0:64, :], ps[64:128, :])  # k2
                # duplicate into the upper partition half (fast SBUF copy)
                nc.vector.tensor_copy(q2d[t][64:128, :], q2d[t][0:64, :])
                nc.vector.tensor_copy(k2d[t][64:128, :], k2d[t][0:64, :])

        def emit_v_group(t, tb):
            # v block tb in [t, d] layout; wv interleaves [v_h | ones] per
            # head. Without bias the ones columns are memset directly.
            psv = mix_ps.tile([128, TQ], f32, tag="mix", name=f"psv_{tb}")
            for j in range(NKT):
                nc.tensor.matmul(
                    psv[0:128, 0:WV_W],
                    xt_c[t][:, TQ * j + TB * (tb % NBL) : TQ * j + TB * (tb % NBL + 1)],
                    wv[j],
                    start=(j == 0), stop=(j == NKT - 1 and not use_bias),
                )
            if use_bias:
                nc.tensor.matmul(
                    psv[0:128, 0:WV_W],
                    xt1[0:1, 0:TB],
                    wv[6][0:1, :],
                    start=False, stop=True,
                )
            nc.vector.tensor_copy(v_sb[tb][:], psv[:, 0:WV_W])
            if not use_bias:
                for h in range(H_LOC):
                    c1 = DV * h + D
                    nc.gpsimd.memset(v_sb[tb][:, c1 : c1 + 1], 1.0)

        def emit_norm_pair(i, po0, po1):
            # row D of po is the softmax denominator. Broadcast both heads'
            # denominators into one [128, TQ] psum via two CONCURRENT
            # col-tiled rank-1 matmuls, one reciprocal, two multiplies.
            d0 = nrm.tile([1, TQ], bf16, tag="d", name=f"d0_{i}")
            d1 = nrm.tile([1, TQ], bf16, tag="d", name=f"d1_{i}")
            nc.vector.tensor_copy(d0[:], po0[D : D + 1, :])
            nc.vector.tensor_copy(d1[:], po1[D : D + 1, :])
            pb = mix_ps.tile([128, TQ], f32, tag="mix", name=f"pbp_{i}")
            nc.tensor.matmul(pb[0:D, :], ones_b[:], d0[:], start=True, stop=True)
            nc.tensor.matmul(pb[D : 2 * D, :], ones_b[:], d1[:],
                             start=True, stop=True)
            rb = nrm.tile([128, TQ], f32, tag="rb", name=f"rbp_{i}")
            nc.vector.reciprocal_approx_fast(rb[:], pb[:])
            nc.vector.tensor_mul(prhs0[i][0:D, :], po0[0:D, :], rb[0:D, :])
            nc.vector.tensor_mul(prhs0[i][D : 2 * D, :], po1[0:D, :],
                                 rb[D : 2 * D, :])

        def emit_norm_h2(i, po):
            d_sb = nrm.tile([1, TQ], bf16, tag="d", name=f"d2_{i}")
            nc.vector.tensor_copy(d_sb[:], po[D : D + 1, :])
            pb = mix_ps.tile([128, TQ], f32, tag="mix", name=f"pb2_{i}")
            nc.tensor.matmul(pb[0:D, :], ones_b[:], d_sb[:],
                             start=True, stop=True)
            rb = nrm.tile([D, TQ], f32, tag="rb2", name=f"rb2_{i}")
            nc.vector.reciprocal_approx_fast(rb[:], pb[0:D, :])
            nc.vector.tensor_mul(prhs1[i][0:64, :], po[0:D, :], rb[:])
            # duplicate h2 rows so the K=64 projection half can row-tile
            nc.vector.tensor_copy(prhs1[i][64:128, :], prhs1[i][0:64, :])

        osb_c = [outp.tile([128, (C // 128) * TQ], bf16, tag=f"osb{i}",
                           name=f"osb{i}", bufs=1) for i in range(NCH)]

        def emit_proj_pair(i, n):
            # projection chunks n, n+1. The K=128 prhs0 halves run as normal
            # full-array matmuls; the two K=64 prhs1 halves are packed into
            # the PE array concurrently via row tiling (wp2/prhs1 hold the
            # same data in both partition halves).
            ppa = mix_ps.tile([128, TQ], f32, tag="mix", name=f"ppa_{i}_{n}")
            ppb = mix_ps.tile([128, TQ], f32, tag="mix", name=f"ppb_{i}_{n}")
            nc.tensor.matmul(ppa[:], wp[:, 128 * n : 128 * (n + 1)],
                             prhs0[i][:], start=True, stop=False)
            nc.tensor.matmul(ppb[:], wp[:, 128 * (n + 1) : 128 * (n + 2)],
                             prhs0[i][:], start=True, stop=False)
            nc.tensor.matmul(ppa[:], wp2[0:64, 128 * n : 128 * (n + 1)],
                             prhs1[i][0:64, :], start=False, stop=True)
            nc.tensor.matmul(ppb[:], wp2[64:128, 128 * (n + 1) : 128 * (n + 2)],
                             prhs1[i][64:128, :], start=False, stop=True)
            for k, pp in ((0, ppa), (1, ppb)):
                dst = osb_c[i][:, TQ * (n + k) : TQ * (n + k + 1)]
                if i == NCH - 1 and k == 1:
                    # tail: ACT is done with exps - split copies across
                    # engines so the last chunk's output drains faster
                    nc.scalar.copy(dst, pp[:])
                else:
                    nc.vector.tensor_copy(dst, pp[:])
            if i == NCH - 1:
                if n + 2 == C // 128:
                    nc.sync.dma_start(out_d[i][:, 4 * TQ :],
                                      osb_c[i][:, 4 * TQ :])
            elif n + 2 == C // 128:
                # all six column chunks written -> one large-row DMA out
                nc.sync.dma_start(out_d[i], osb_c[i][:])

        # ------------------------------------------------------------------
        # Attention units, software-pipelined: iteration u emits S(u) on the
        # PE queue, THEN [exp, mask, PV] of unit u-1, then a filler piece.
        # S(u) therefore always runs during exp(u-1) and the ACT exp stream
        # never waits on PV/filler work queued behind it.
        # ------------------------------------------------------------------
        po_t = {}

        def emit_S(u):
            kind, i, p = u
            if kind == "pair":
                j = p - NBL * i
                c0 = 0 if j < 0 else TB * j
                ps2 = s_ps.tile([128, 2 * TQ], f32, tag="s", name=f"sp_{i}_{p}")
                blk = slice(TB * (p % NBL), TB * (p % NBL + 1))
                nc.tensor.matmul(ps2[:, c0:TQ],
                                 kAB[p // NBL][0:64, blk],
                                 qAB[i][0:64, c0:TQ], start=True, stop=True)
                nc.tensor.matmul(ps2[:, TQ + c0 : 2 * TQ],
                                 kAB[p // NBL][64:128, blk],
                                 qAB[i][64:128, c0:TQ], start=True, stop=True)
                return (ps2, (c0, c0))
            else:
                ps2 = s_ps.tile([128, 2 * TQ], f32, tag="s", name=f"s2_{i}_{p}")
                c0s = []
                for half in range(2):
                    Bq = 2 * p + half
                    j = Bq - NBL * i
                    c0 = 0 if j < 0 else TB * j
                    c0s.append(c0)
                    off = TQ * half
                    nc.tensor.matmul(
                        ps2[:, off + c0 : off + TQ],
                        k2d[Bq // NBL][64 * half : 64 * (half + 1),
                                       TB * (Bq % NBL) : TB * (Bq % NBL + 1)],
                        q2d[i][64 * half : 64 * (half + 1), c0:TQ],
                        start=True, stop=True,
                    )
                return (ps2, tuple(c0s))

        def emit_rest(u, ps2, c0s):
            kind, i, p = u
            nblk = NBL * (i + 1)
            pt = pt_p.tile([128, 2 * TQ], bf16, tag="pt", name=f"pt_{kind}_{i}_{p}")
            nc.scalar.activation(pt[:, c0s[0] :], ps2[:, c0s[0] :], EXP)
            if kind == "pair":
                if p == 0:
                    po_t[(i, 0)] = po_ps.tile([DV, TQ], f32, tag="po",
                                              name=f"po0_{i}")
                    po_t[(i, 1)] = po_ps.tile([DV, TQ], f32, tag="po",
                                              name=f"po1_{i}")
                j = p - NBL * i
                c0 = c0s[0]
                for half in range(2):
                    off = TQ * half
                    if j >= 0:
                        nc.gpsimd.affine_select(
                            pt[:, off + TB * j : off + TB * (j + 1)],
                            pt[:, off + TB * j : off + TB * (j + 1)],
                            pattern=[[1, TB]],
                            compare_op=mybir.AluOpType.is_ge,
                            fill=0.0,
                            base=0,
                            channel_multiplier=-1,
                        )
                    nc.tensor.matmul(
                        po_t[(i, half)][:, c0:TQ],
                        v_sb[p][:, DV * half : DV * (half + 1)],
                        pt[:, off + c0 : off + TQ],
                        start=(p == 0), stop=(p == nblk - 1),
                    )
                if p == nblk - 1:
                    emit_norm_pair(i, po_t[(i, 0)], po_t[(i, 1)])
            else:
                if p == 0:
                    po_t[(i, 2)] = po_ps.tile([DV, TQ], f32, tag="po",
                                              name=f"po2_{i}")
                for half in range(2):
                    Bq = 2 * p + half
                    j = Bq - NBL * i
                    c0 = c0s[half]
                    off = TQ * half
                    if j >= 0:
                        nc.gpsimd.affine_select(
                            pt[:, off + TB * j : off + TB * (j + 1)],
                            pt[:, off + TB * j : off + TB * (j + 1)],
                            pattern=[[1, TB]],
                            compare_op=mybir.AluOpType.is_ge,
                            fill=0.0,
                            base=0,
                            channel_multiplier=-1,
                        )
                    nc.tensor.matmul(
                        po_t[(i, 2)][:, c0:TQ],
                        v_sb[Bq][:, 2 * DV : 3 * DV],
                        pt[:, off + c0 : off + TQ],
                        start=(Bq == 0), stop=(Bq == nblk - 1),
                    )
                if p == nblk // 2 - 1:
                    emit_norm_h2(i, po_t[(i, 2)])

        def run_piece(piece):
            kind, a, b = piece
            if kind == "qk":
                emit_qk_group(a, b)
            elif kind == "v":
                emit_v_group(a, b)
            else:
                emit_proj_pair(a, b)

        # Filler pieces with DEADLINES (latest unit index at which the piece
        # must be emitted so its consumer's dependency order is correct) and
        # PE costs. Pieces pop when the accrued slack budget covers them, or
        # when forced by their deadline - this spreads PE filler work into
        # the exp-bound attention stream instead of bursting it.
        units = []
        for i in range(NCH):
            nblk = NBL * (i + 1)
            units += [("pair", i, p) for p in range(nblk)]
            units += [("h2", i, p) for p in range(nblk // 2)]
        first_u = {i: units.index(("pair", i, 0)) for i in range(NCH)}
        first_h2 = {i: units.index(("h2", i, 0)) for i in range(NCH)}

        def vdl(t, tb):
            return first_u[t] + tb - NBL * t

        plan = {
            0: [(vdl(0, tb), 550, ("v", 0, tb)) for tb in range(NBL)] +
               [(first_u[1] - 1, 1300, ("qk", 1, 0)),
                (first_u[1] - 1, 1300, ("qk", 1, 1)),
                (first_h2[1] - 1, 1300, ("qk", 1, 2))],
            1: [(vdl(1, tb), 550, ("v", 1, tb)) for tb in range(NBL, 2 * NBL)] +
               [(first_u[2] - 1, 1300, ("qk", 2, 0)),
                (first_u[2] - 1, 1300, ("qk", 2, 1)),
                (first_h2[2] - 1, 1300, ("qk", 2, 2))] +
               [(vdl(2, tb), 550, ("v", 2, tb))
                for tb in range(2 * NBL, 3 * NBL)],
            2: [(vdl(3, tb), 550, ("v", 3, tb))
                for tb in range(3 * NBL, 4 * NBL)] +
               [(first_u[3] - 1, 1300, ("qk", 3, 0)),
                (first_u[3] - 1, 1300, ("qk", 3, 1)),
                (first_h2[3] - 1, 1300, ("qk", 3, 2))],
            3: [(10 ** 6, 750, ("proj", i, n)) for i in range(NCH - 1)
                for n in range(0, C // 128, 2)],
        }
        nu = {i: sum(1 for u in units if u[1] == i) for i in range(NCH)}
        rate = {i: sum(c for _, c, _ in plan[i]) / nu[i] for i in range(NCH)}

        # chunk-0 q/k production before the pipeline starts
        for m in range(3):
            emit_qk_group(0, m)

        budget = 0.0
        pending = None
        for u_idx, u in enumerate(units):
            sctx = emit_S(u)
            if pending is not None:
                emit_rest(*pending)
            i = u[1]
            budget += rate[i] + 1.0
            for t in range(i + 1):          # earlier chunks' leftovers first
                q = plan[t]
                while q:
                    dl, cost, (kind, a, b) = q[0]
                    if kind == "proj" and i <= a:
                        break
                    if dl > u_idx and budget < cost:
                        break
                    budget = max(0.0, budget - cost)
                    run_piece(q.pop(0)[2])
            pending = (u, *sctx)
        emit_rest(*pending)
        for i in range(NCH):
            while plan[i]:
                run_piece(plan[i].pop(0)[2])
        # --- chunk-3 projection tail. The s-pool psum banks are free after
        # the final exp: run the four K=128 prhs0 halves there immediately
        # (they only need the pair normalize, done ~10 units earlier) so the
        # PE works while the h2 normalize chain runs on DVE; only the small
        # row-tiled K=64 halves + copies + DMA remain at the very end.
        lc = NCH - 1
        s1 = s_ps.tile([128, 2 * TQ], f32, tag="s", name="pp1a")
        s2 = s_ps.tile([128, 2 * TQ], f32, tag="s", name="pp1b")
        slots = [s1[:, 0:TQ], s1[:, TQ:], s2[:, 0:TQ], s2[:, TQ:]]
        for n in range(4):
            nc.tensor.matmul(slots[n], wp[:, 128 * n : 128 * (n + 1)],
                             prhs0[lc][:], start=True, stop=False)
        for base in (0, 2):
            nc.tensor.matmul(slots[base], wp2[0:64, 128 * base : 128 * (base + 1)],
                             prhs1[lc][0:64, :], start=False, stop=True)
            nc.tensor.matmul(slots[base + 1],
                             wp2[64:128, 128 * (base + 1) : 128 * (base + 2)],
                             prhs1[lc][64:128, :], start=False, stop=True)
        for n in range(4):
            dst = osb_c[lc][:, TQ * n : TQ * (n + 1)]
            if n % 2:
                nc.scalar.copy(dst, slots[n])
            else:
                nc.vector.tensor_copy(dst, slots[n])
        nc.sync.dma_start(out_d[lc][:, 0 : 4 * TQ], osb_c[lc][:, 0 : 4 * TQ])
        emit_proj_pair(lc, 4)

    nc.compile()
    return nc


_PROG_CACHE = {}


def kernel(x, Wqkv, bqkv, Wproj, bproj):
    global LAST_RESULT
    x = np.asarray(x, dtype=np.float32)
    Wqkv = np.asarray(Wqkv, dtype=np.float32)
    bqkv = np.asarray(bqkv, dtype=np.float32)
    Wproj = np.asarray(Wproj, dtype=np.float32)
    bproj = np.asarray(bproj, dtype=np.float32)

    Wq, Wk, Wv = Wqkv[:, 0:C], Wqkv[:, C : 2 * C], Wqkv[:, 2 * C : 3 * C]
    bq, bk, bv = bqkv[0:C], bqkv[C : 2 * C], bqkv[2 * C : 3 * C]
    scale = 1.0 / np.sqrt(D)

    use_bias = bool(np.any(bq) or np.any(bk) or np.any(bv))
    if use_bias not in _PROG_CACHE:
        _PROG_CACHE[use_bias] = _build_program(use_bias)
    nc = _PROG_CACHE[use_bias]

    in_maps = []
    for c in range(N_CORES):
        b = c // (N_CORES // B)
        g = c % (N_CORES // B)
        hs = slice(DL * g, DL * (g + 1))       # this core's head-dim rows/cols

        # x^T packed per (chunk, k-tile): [NCH, 128, NKT*TQ]
        xt = np.ascontiguousarray(
            x[b].T.reshape(NKT, 128, NCH, TQ).transpose(2, 1, 0, 3)
        ).reshape(NCH, 128, NKT * TQ)
        xt1 = np.ones((1, TQ), np.float32)

        wq_loc = Wq[:, hs] * scale             # fold 1/sqrt(D) into q
        bq_loc = bq[hs] * scale
        wk_loc, bk_loc = Wk[:, hs], bk[hs]
        wv_loc, bv_loc = Wv[:, hs], bv[hs]

        wqk = np.zeros((C + 128, 2 * DL), np.float32)   # 7 k-tiles of 128
        wqk[0:C, 0:128] = wq_loc[:, 0:128]
        wqk[C, 0:128] = bq_loc[0:128]
        wqk[0:C, 128:256] = wk_loc[:, 0:128]
        wqk[C, 128:256] = bk_loc[0:128]
        wqk[0:C, 256:320] = wq_loc[:, 128:192]
        wqk[C, 256:320] = bq_loc[128:192]
        wqk[0:C, 320:384] = wk_loc[:, 128:192]
        wqk[C, 320:384] = bk_loc[128:192]
        # m-major: [m, j, 128] per partition row (production group m only
        # needs its own contiguous 896-column slice)
        wqk = np.concatenate(
            [np.ascontiguousarray(
                wqk[:, 128 * m : 128 * (m + 1)].reshape(7, 128, 128)
                .transpose(1, 0, 2)).reshape(128, 896)
             for m in range(3)], axis=1)

        wv_pad = np.zeros((C + 128, WV_W), np.float32)
        for h in range(H_LOC):
            c0 = DV * h
            wv_pad[0:C, c0 : c0 + D] = wv_loc[:, D * h : D * (h + 1)]
            wv_pad[C, c0 : c0 + D] = bv_loc[D * h : D * (h + 1)]
            wv_pad[C, c0 + D] = 1.0            # ones column -> softmax denom
        wv_pad = np.ascontiguousarray(
            wv_pad.reshape(7, 128, WV_W).transpose(1, 0, 2)).reshape(128, -1)

        wp = np.zeros((2, 128, C), np.float32)
        wp[0] = Wproj[DL * g : DL * g + 128, :]  # cast to bf16 below
        wp[1, 0:64] = Wproj[DL * g + 128 : DL * (g + 1), :]
        wp[1, 64:128] = wp[1, 0:64]              # dup for row-tiled proj half
        wp = np.ascontiguousarray(wp.transpose(1, 0, 2)).reshape(128, 2 * C)

        bf = ml_dtypes.bfloat16
        in_maps.append({"xt": xt.astype(bf), "xt1": xt1.astype(bf),
                        "wqk": wqk.astype(bf), "wv": wv_pad.astype(bf),
                        "wp": wp.astype(bf)})

    res = bass_utils.run_bass_kernel_spmd(nc, in_maps, core_ids=list(range(N_CORES)))
    LAST_RESULT = res

    out = np.zeros((B, T, C), np.float32)
    for c in range(N_CORES):
        b = c // (N_CORES // B)
        # outc [i, 128, n*512] -> [C, T] -> [T, C]
        outT = (res.results[c]["outc"].astype(np.float32)
                .reshape(NCH, 128, C // 128, TQ)
                .transpose(2, 1, 0, 3).reshape(C, T))
        out[b] += outT.T
    return out + bproj


if __name__ == "__main__":
    rng = np.random.default_rng(0)
    s = 1.0 / np.sqrt(C)
    ins = {
        "x": rng.standard_normal((B, T, C), dtype=np.float32),
        "Wqkv": rng.standard_normal((C, 3 * C), dtype=np.float32) * s,
        "bqkv": np.zeros(3 * C, np.float32),
        "Wproj": rng.standard_normal((C, C), dtype=np.float32) * s,
        "bproj": np.zeros(C, np.float32),
    }
    out = kernel(**ins)
    print("out", out.shape, out.dtype, float(np.abs(out).max()))


# revision 32
# speedup vs baseline: 1.1462x; 1.0180x over previous
"""Multi-head causal self-attention (B=2, T=2048, C=768, H=12, D=64) on 8
Trainium2 NeuronCores.

Sharding: 24 (batch, head) units -> 3 heads per core; cores 0-3 take batch 0,
cores 4-7 take batch 1. Each core computes q/k/v projections for its 3 heads,
flash-style causal attention fully on-chip (no T x T tensor ever touches HBM),
and a partial output projection with its 192-row slice of Wproj. The host sums
the 4 partial projections per batch.

Device design notes:
  - The attention S^T = K^T.T @ Q^T matmuls have a K=64 contraction, so they
    are packed two-at-a-time into the 128x128 PE array via row tiling
    (tile_position (0,0)/(64,0) run CONCURRENTLY): heads 0 and 1 share one
    [q0;q1]/[k0;k1] tile pair and compute the same tk-block together; head 2
    uses duplicated [q2;q2]/[k2;k2] tiles and computes two consecutive
    tk-blocks together.
  - The kernel is ACT(exp)-bound: ~6.7M exp elements per core at 1 elem/
    lane/cycle @ 1.2 GHz is ~50us. The emission is SOFTWARE-PIPELINED so the
    exp stream never stalls on the in-order PE queue: for attention unit u,
    S(u) is emitted BEFORE [exp, mask, PV] of unit u-1, so S(u) always
    completes while exp(u-1) runs and exp(u) can start back-to-back.
    Production/projection fillers are emitted after each PV in small pieces.
  - V is augmented with a ones column per head: PV accumulation yields the
    softmax denominator as psum row 64 for free. Causal masking: matmul
    columns restricted to tq >= tk-block start; the diagonal 128x128
    sub-block is zeroed above the diagonal via affine_select (GpSimd) after
    exp (PV consumes pt afterwards; exp garbage in never-read columns is
    harmless).
  - Inputs arrive as a few large-row DMAs (6KB per partition for x chunks)
    instead of many 1KB-row transfers - DMA engines process ~2x faster.
  - A warm-up burst of matmuls at t=0 flips the PE HAM clock gate to 2.4GHz
    during the input-DMA window; a dummy exp preloads the ACT spline table.
  - Output is written in chunked [i, n, 128, 512] layout (contiguous DMA);
    host reassembles and reduces.
"""

import os
import sys

sys.path.insert(0, "/opt/trn_rl_repo")

import ml_dtypes
import numpy as np

import concourse.bass as bass
import concourse.tile as tile
from concourse import bacc, mybir
from concourse import bass_utils

B, T, C = 2, 2048, 768
H, D = 12, 64
N_CORES = 8
H_LOC = 3           # heads per core
DL = H_LOC * D      # 192 local head dims
TQ = 512            # tq chunk (psum bank width)
TB = 128            # tk block
NCH = T // TQ       # 4 chunks
NBL = TQ // TB      # 4 blocks per chunk
NKT = C // 128      # 6 contraction k-tiles
DV = D + 1          # head dim + denominator column
WV_W = H_LOC * DV   # 195 packed v columns
NWARM = 28          # HAM warm-up matmuls (N=256): stay dense until x0 lands

f32 = mybir.dt.float32
bf16 = mybir.dt.bfloat16
EXP = mybir.ActivationFunctionType.Exp

LAST_RESULT = None  # test harness reads exec_time_ns from here


def _build_program(use_bias: bool):
    from contextlib import ExitStack

    nc = bacc.Bacc("TRN2", target_bir_lowering=False, debug=False,
                   num_devices=N_CORES)

    xt_d = nc.dram_tensor("xt", [NCH, 128, NKT * TQ], bf16, kind="ExternalInput").ap()
    xt1_d = nc.dram_tensor("xt1", [1, TQ], bf16, kind="ExternalInput").ap()
    wqk_d = nc.dram_tensor("wqk", [128, 7 * 2 * DL], bf16, kind="ExternalInput").ap()
    wv_d = nc.dram_tensor("wv", [128, 7 * WV_W], bf16, kind="ExternalInput").ap()
    wp_d = nc.dram_tensor("wp", [128, 2 * C], bf16, kind="ExternalInput").ap()
    out_d = nc.dram_tensor("outc", [NCH, 128, (C // 128) * TQ], bf16,
                           kind="ExternalOutput").ap()

    with tile.TileContext(nc) as tc, ExitStack() as ctx:
        cpool = ctx.enter_context(tc.tile_pool(name="const", bufs=1))
        wpool = ctx.enter_context(tc.tile_pool(name="w", bufs=1))
        xpool = ctx.enter_context(tc.tile_pool(name="x", bufs=1))
        qkpool = ctx.enter_context(tc.tile_pool(name="qk", bufs=1))

        # PSUM budget (8 banks): s 2x2 + po 2x1 + mix 2x1.
        s_ps = ctx.enter_context(tc.tile_pool(name="s_ps", bufs=2, space="PSUM"))
        po_ps = ctx.enter_context(tc.tile_pool(name="po_ps", bufs=2, space="PSUM"))
        mix_ps = ctx.enter_context(tc.tile_pool(name="mix_ps", bufs=2, space="PSUM"))
        pt_p = ctx.enter_context(tc.tile_pool(name="pt_p", bufs=10))
        nrm = ctx.enter_context(tc.tile_pool(name="nrm", bufs=4))
        outp = ctx.enter_context(tc.tile_pool(name="outp", bufs=5))

        ones_b = cpool.tile([1, D], bf16)
        nc.vector.memset(ones_b[:], 1.0)

        # --- HAM warm-up: keep the PE busy from t=0 so the clock gate flips
        # to 2.4 GHz during the input-DMA window, not 15us into compute.
        wmt = cpool.tile([128, TQ], bf16)
        nc.vector.memset(wmt[:], 0.0)
        for w in range(NWARM):
            wps = mix_ps.tile([128, TQ], f32, tag="mix", name=f"warm{w}")
            nc.tensor.matmul(wps[:, 0:256], wmt[:, 0:128], wmt[:, 0:256],
                             start=True, stop=True)
        # ACT spline-table preload (~2.7us) off the critical path.
        actw = nrm.tile([128, 1], f32, tag="actw", name="actw")
        nc.scalar.activation(actw[:], wmt[:, 0:1], EXP)

        # --- input loads: one large-row DMA per tensor / x chunk. x chunk 0
        # goes FIRST (it gates the first production), wqk in m-major slices
        # right after (production group m needs only its own slice).
        xt_c = []
        for t in range(NCH):
            t_ = xpool.tile([128, NKT * TQ], bf16, tag=f"xtc{t}", name=f"xtc{t}")
            xt_c.append(t_)
        hw = NKT * TQ // 2
        nc.sync.dma_start(xt_c[0][:, 0:hw], xt_d[0, :, 0:hw])
        nc.sync.dma_start(xt_c[0][:, hw:], xt_d[0, :, hw:])
        wqk_t = wpool.tile([128, 7 * 2 * DL], bf16)
        for m in range(3):
            nc.sync.dma_start(wqk_t[:, 896 * m : 896 * (m + 1)],
                              wqk_d[:, 896 * m : 896 * (m + 1)])

        def wqk_mj(m, j):
            return wqk_t[:, 896 * m + 128 * j : 896 * m + 128 * (j + 1)]

        wv_t = wpool.tile([128, 7 * WV_W], bf16)
        nc.sync.dma_start(wv_t[:], wv_d[:])
        wv = [wv_t[:, WV_W * j : WV_W * (j + 1)] for j in range(7)]
        for t in range(1, NCH):
            nc.sync.dma_start(xt_c[t][:], xt_d[t])
        if use_bias:
            xt1 = xpool.tile([1, TQ], bf16)
            nc.sync.dma_start(xt1[:], xt1_d[:])
        wp_t = wpool.tile([128, 2 * C], bf16)
        nc.sync.dma_start(wp_t[:], wp_d[:])
        wp = wp_t[:, 0:C]
        wp2 = wp_t[:, C : 2 * C]      # rows 64-127 are zeros (host pads)

        def xt(t, j):
            return xt_c[t][:, TQ * j : TQ * (j + 1)]

        # Attention-stage tiles (bf16). Heads 0/1 share [q0;q1]/[k0;k1]
        # tiles (the row-tiled S pair reads partition halves 0:64 / 64:128);
        # head 2 gets duplicated [q2;q2]/[k2;k2] so two consecutive
        # tk-blocks can run concurrently.
        qAB, kAB, q2d, k2d = [], [], [], []
        for t in range(NCH):
            qAB.append(qkpool.tile([128, TQ], bf16, tag=f"qAB{t}", name=f"qAB{t}"))
            kAB.append(qkpool.tile([128, TQ], bf16, tag=f"kAB{t}", name=f"kAB{t}"))
            q2d.append(qkpool.tile([128, TQ], bf16, tag=f"q2d{t}", name=f"q2d{t}"))
            k2d.append(qkpool.tile([128, TQ], bf16, tag=f"k2d{t}", name=f"k2d{t}"))
        v_sb = [qkpool.tile([128, WV_W], bf16, tag=f"v{t}", name=f"v{t}")
                for t in range(T // TB)]
        # per-chunk normalized-O^T tiles (per-tile deps: deferred proj of
        # chunk t-1 must not wait on chunk t's normalize)
        prhs0 = [qkpool.tile([128, TQ], bf16, tag=f"prhs0{t}", name=f"prhs0{t}")
                 for t in range(NCH)]      # heads 0,1
        prhs1 = [qkpool.tile([128, TQ], bf16, tag=f"prhs1{t}", name=f"prhs1{t}")
                 for t in range(NCH)]      # head 2, duplicated in both halves

        def emit_qk_group(t, m):
            # chunk t of q^T/k^T; M-tiles: [q0|q1], [k0|k1], [q2|k2]
            ps = mix_ps.tile([128, TQ], f32, tag="mix", name=f"ps_{t}_{m}")
            for j in range(NKT):
                nc.tensor.matmul(
                    ps[:],
                    wqk_mj(m, j),
                    xt(t, j),
                    start=(j == 0),
                    stop=(j == NKT - 1 and not use_bias),
                )
            if use_bias:
                nc.tensor.matmul(
                    ps[:], wqk_mj(m, 6)[0:1, :],
                    xt1[:], start=False, stop=True,
                )
            if m == 0:
                nc.vector.tensor_copy(qAB[t][:], ps[:])                # q0;q1
            elif m == 1:
                nc.vector.tensor_copy(kAB[t][:], ps[:])                # k0;k1
            else:
                nc.vector.tensor_copy(q2d[t][0:64, :], ps[0:64, :])    # q2
                nc.vector.tensor_copy(k2d[t][0:64, :], ps[64:128, :])  # k2
                # duplicate into the upper partition half (fast SBUF copy)
                nc.vector.tensor_copy(q2d[t][64:128, :], q2d[t][0:64, :])
                nc.vector.tensor_copy(k2d[t][64:128, :], k2d[t][0:64, :])

        def emit_v_group(t, tb):
            # v block tb in [t, d] layout; wv interleaves [v_h | ones] per
            # head. Without bias the ones columns are memset directly.
            psv = mix_ps.tile([128, TQ], f32, tag="mix", name=f"psv_{tb}")
            for j in range(NKT):
                nc.tensor.matmul(
                    psv[0:128, 0:WV_W],
                    xt_c[t][:, TQ * j + TB * (tb % NBL) : TQ * j + TB * (tb % NBL + 1)],
                    wv[j],
                    start=(j == 0), stop=(j == NKT - 1 and not use_bias),
                )
            if use_bias:
                nc.tensor.matmul(
                    psv[0:128, 0:WV_W],
                    xt1[0:1, 0:TB],
                    wv[6][0:1, :],
                    start=False, stop=True,
                )
            nc.vector.tensor_copy(v_sb[tb][:], psv[:, 0:WV_W])
            if not use_bias:
                for h in range(H_LOC):
                    c1 = DV * h + D
                    nc.gpsimd.memset(v_sb[tb][:, c1 : c1 + 1], 1.0)

        def emit_norm_pair(i, po0, po1):
            # row D of po is the softmax denominator. Broadcast both heads'
            # denominators into one [128, TQ] psum via two CONCURRENT
            # col-tiled rank-1 matmuls, one reciprocal, two multiplies.
            d0 = nrm.tile([1, TQ], bf16, tag="d", name=f"d0_{i}")
            d1 = nrm.tile([1, TQ], bf16, tag="d", name=f"d1_{i}")
            nc.vector.tensor_copy(d0[:], po0[D : D + 1, :])
            nc.vector.tensor_copy(d1[:], po1[D : D + 1, :])
            pb = mix_ps.tile([128, TQ], f32, tag="mix", name=f"pbp_{i}")
            nc.tensor.matmul(pb[0:D, :], ones_b[:], d0[:], start=True, stop=True)
            nc.tensor.matmul(pb[D : 2 * D, :], ones_b[:], d1[:],
                             start=True, stop=True)
            rb = nrm.tile([128, TQ], f32, tag="rb", name=f"rbp_{i}")
            nc.vector.reciprocal_approx_fast(rb[:], pb[:])
            nc.vector.tensor_mul(prhs0[i][0:D, :], po0[0:D, :], rb[0:D, :])
            nc.vector.tensor_mul(prhs0[i][D : 2 * D, :], po1[0:D, :],
                                 rb[D : 2 * D, :])

        def emit_norm_h2(i, po):
            d_sb = nrm.tile([1, TQ], bf16, tag="d", name=f"d2_{i}")
            nc.vector.tensor_copy(d_sb[:], po[D : D + 1, :])
            pb = mix_ps.tile([128, TQ], f32, tag="mix", name=f"pb2_{i}")
            nc.tensor.matmul(pb[0:D, :], ones_b[:], d_sb[:],
                             start=True, stop=True)
            rb = nrm.tile([D, TQ], f32, tag="rb2", name=f"rb2_{i}")
            nc.vector.reciprocal_approx_fast(rb[:], pb[0:D, :])
            nc.vector.tensor_mul(prhs1[i][0:64, :], po[0:D, :], rb[:])
            # duplicate h2 rows so the K=64 projection half can row-tile
            nc.vector.tensor_copy(prhs1[i][64:128, :], prhs1[i][0:64, :])

        osb_c = [outp.tile([128, (C // 128) * TQ], bf16, tag=f"osb{i}",
                           name=f"osb{i}", bufs=1) for i in range(NCH)]

        def emit_proj_pair(i, n):
            # projection chunks n, n+1. The K=128 prhs0 halves run as normal
            # full-array matmuls; the two K=64 prhs1 halves are packed into
            # the PE array concurrently via row tiling (wp2/prhs1 hold the
            # same data in both partition halves).
            ppa = mix_ps.tile([128, TQ], f32, tag="mix", name=f"ppa_{i}_{n}")
            ppb = mix_ps.tile([128, TQ], f32, tag="mix", name=f"ppb_{i}_{n}")
            nc.tensor.matmul(ppa[:], wp[:, 128 * n : 128 * (n + 1)],
                             prhs0[i][:], start=True, stop=False)
            nc.tensor.matmul(ppb[:], wp[:, 128 * (n + 1) : 128 * (n + 2)],
                             prhs0[i][:], start=True, stop=False)
            nc.tensor.matmul(ppa[:], wp2[0:64, 128 * n : 128 * (n + 1)],
                             prhs1[i][0:64, :], start=False, stop=True)
            nc.tensor.matmul(ppb[:], wp2[64:128, 128 * (n + 1) : 128 * (n + 2)],
                             prhs1[i][64:128, :], start=False, stop=True)
            for k, pp in ((0, ppa), (1, ppb)):
                dst = osb_c[i][:, TQ * (n + k) : TQ * (n + k + 1)]
                if i == NCH - 1 and k == 1:
                    # tail: ACT is done with exps - split copies across
                    # engines so the last chunk's output drains faster
                    nc.scalar.copy(dst, pp[:])
                else:
                    nc.vector.tensor_copy(dst, pp[:])
            if i == NCH - 1:
                if n + 2 == C // 128:
                    nc.sync.dma_start(out_d[i][:, 4 * TQ :],
                                      osb_c[i][:, 4 * TQ :])
            elif n + 2 == C // 128:
                # all six column chunks written -> one large-row DMA out
                nc.sync.dma_start(out_d[i], osb_c[i][:])

        # ------------------------------------------------------------------
        # Attention units, software-pipelined: iteration u emits S(u) on the
        # PE queue, THEN [exp, mask, PV] of unit u-1, then a filler piece.
        # S(u) therefore always runs during exp(u-1) and the ACT exp stream
        # never waits on PV/filler work queued behind it.
        # ------------------------------------------------------------------
        po_t = {}

        def emit_S(u):
            kind, i, p = u
            if kind == "pair":
                j = p - NBL * i
                c0 = 0 if j < 0 else TB * j
                ps2 = s_ps.tile([128, 2 * TQ], f32, tag="s", name=f"sp_{i}_{p}")
                blk = slice(TB * (p % NBL), TB * (p % NBL + 1))
                nc.tensor.matmul(ps2[:, c0:TQ],
                                 kAB[p // NBL][0:64, blk],
                                 qAB[i][0:64, c0:TQ], start=True, stop=True)
                nc.tensor.matmul(ps2[:, TQ + c0 : 2 * TQ],
                                 kAB[p // NBL][64:128, blk],
                                 qAB[i][64:128, c0:TQ], start=True, stop=True)
                return (ps2, (c0, c0))
            else:
                ps2 = s_ps.tile([128, 2 * TQ], f32, tag="s", name=f"s2_{i}_{p}")
                c0s = []
                for half in range(2):
                    Bq = 2 * p + half
                    j = Bq - NBL * i
                    c0 = 0 if j < 0 else TB * j
                    c0s.append(c0)
                    off = TQ * half
                    nc.tensor.matmul(
                        ps2[:, off + c0 : off + TQ],
                        k2d[Bq // NBL][64 * half : 64 * (half + 1),
                                       TB * (Bq % NBL) : TB * (Bq % NBL + 1)],
                        q2d[i][64 * half : 64 * (half + 1), c0:TQ],
                        start=True, stop=True,
                    )
                return (ps2, tuple(c0s))

        def emit_rest(u, ps2, c0s):
            kind, i, p = u
            nblk = NBL * (i + 1)
            pt = pt_p.tile([128, 2 * TQ], bf16, tag="pt", name=f"pt_{kind}_{i}_{p}")
            nc.scalar.activation(pt[:, c0s[0] :], ps2[:, c0s[0] :], EXP)
            if kind == "pair":
                if p == 0:
                    po_t[(i, 0)] = po_ps.tile([DV, TQ], f32, tag="po",
                                              name=f"po0_{i}")
                    po_t[(i, 1)] = po_ps.tile([DV, TQ], f32, tag="po",
                                              name=f"po1_{i}")
                j = p - NBL * i
                c0 = c0s[0]
                for half in range(2):
                    off = TQ * half
                    if j >= 0:
                        nc.gpsimd.affine_select(
                            pt[:, off + TB * j : off + TB * (j + 1)],
                            pt[:, off + TB * j : off + TB * (j + 1)],
                            pattern=[[1, TB]],
                            compare_op=mybir.AluOpType.is_ge,
                            fill=0.0,
                            base=0,
                            channel_multiplier=-1,
                        )
                    nc.tensor.matmul(
                        po_t[(i, half)][:, c0:TQ],
                        v_sb[p][:, DV * half : DV * (half + 1)],
                        pt[:, off + c0 : off + TQ],
                        start=(p == 0), stop=(p == nblk - 1),
                    )
                if p == nblk - 1:
                    emit_norm_pair(i, po_t[(i, 0)], po_t[(i, 1)])
            else:
                if p == 0:
                    po_t[(i, 2)] = po_ps.tile([DV, TQ], f32, tag="po",
                                              name=f"po2_{i}")
                for half in range(2):
                    Bq = 2 * p + half
                    j = Bq - NBL * i
                    c0 = c0s[half]
                    off = TQ * half
                    if j >= 0:
                        nc.gpsimd.affine_select(
                            pt[:, off + TB * j : off + TB * (j + 1)],
                            pt[:, off + TB * j : off + TB * (j + 1)],
                            pattern=[[1, TB]],
                            compare_op=mybir.AluOpType.is_ge,
                            fill=0.0,
                            base=0,
                            channel_multiplier=-1,
                        )
                    nc.tensor.matmul(
                        po_t[(i, 2)][:, c0:TQ],
                        v_sb[Bq][:, 2 * DV : 3 * DV],
                        pt[:, off + c0 : off + TQ],
                        start=(Bq == 0), stop=(Bq == nblk - 1),
                    )
                if p == nblk // 2 - 1:
                    emit_norm_h2(i, po_t[(i, 2)])

        def run_piece(piece):
            kind, a, b = piece
            if kind == "qk":
                emit_qk_group(a, b)
            elif kind == "v":
                emit_v_group(a, b)
            else:
                emit_proj_pair(a, b)

        # Filler pieces with DEADLINES (latest unit index at which the piece
        # must be emitted so its consumer's dependency order is correct) and
        # PE costs. Pieces pop when the accrued slack budget covers them, or
        # when forced by their deadline - this spreads PE filler work into
        # the exp-bound attention stream instead of bursting it.
        units = []
        for i in range(NCH):
            nblk = NBL * (i + 1)
            units += [("pair", i, p) for p in range(nblk)]
            units += [("h2", i, p) for p in range(nblk // 2)]
        first_u = {i: units.index(("pair", i, 0)) for i in range(NCH)}
        first_h2 = {i: units.index(("h2", i, 0)) for i in range(NCH)}

        def vdl(t, tb):
            return first_u[t] + tb - NBL * t

        plan = {
            0: [(vdl(0, tb), 550, ("v", 0, tb)) for tb in range(NBL)] +
               [(first_u[1] - 1, 1300, ("qk", 1, 0)),
                (first_u[1] - 1, 1300, ("qk", 1, 1)),
                (first_h2[1] - 1, 1300, ("qk", 1, 2))],
            1: [(vdl(1, tb), 550, ("v", 1, tb)) for tb in range(NBL, 2 * NBL)] +
               [(first_u[2] - 1, 1300, ("qk", 2, 0)),
                (first_u[2] - 1, 1300, ("qk", 2, 1)),
                (first_h2[2] - 1, 1300, ("qk", 2, 2))] +
               [(vdl(2, tb), 550, ("v", 2, tb))
                for tb in range(2 * NBL, 3 * NBL)],
            2: [(vdl(3, tb), 550, ("v", 3, tb))
                for tb in range(3 * NBL, 4 * NBL)] +
               [(first_u[3] - 1, 1300, ("qk", 3, 0)),
                (first_u[3] - 1, 1300, ("qk", 3, 1)),
                (first_h2[3] - 1, 1300, ("qk", 3, 2))],
            3: [(first_u[3] + 3 + 2 * k, 750, ("proj", i, n))
                for k, (i, n) in enumerate((i, n) for i in range(NCH - 1)
                                           for n in range(0, C // 128, 2))],
        }
        nu = {i: sum(1 for u in units if u[1] == i) for i in range(NCH)}
        rate = {i: sum(c for _, c, _ in plan[i]) / nu[i] for i in range(NCH)}

        # chunk-0 q/k production before the pipeline starts
        for m in range(3):
            emit_qk_group(0, m)

        budget = 0.0
        pending = None
        for u_idx, u in enumerate(units):
            sctx = emit_S(u)
            if pending is not None:
                emit_rest(*pending)
            i = u[1]
            budget += rate[i] + 1.0
            for t in range(i + 1):          # earlier chunks' leftovers first
                q = plan[t]
                while q:
                    dl, cost, (kind, a, b) = q[0]
                    if kind == "proj" and i <= a:
                        break
                    if dl > u_idx and budget < cost:
                        break
                    budget = max(0.0, budget - cost)
                    run_piece(q.pop(0)[2])
            pending = (u, *sctx)
        emit_rest(*pending)
        for i in range(NCH):
            while plan[i]:
                run_piece(plan[i].pop(0)[2])
        # --- chunk-3 projection tail. The s-pool psum banks are free after
        # the final exp: run the four K=128 prhs0 halves there immediately
        # (they only need the pair normalize, done ~10 units earlier) so the
        # PE works while the h2 normalize chain runs on DVE; only the small
        # row-tiled K=64 halves + copies + DMA remain at the very end.
        lc = NCH - 1
        s1 = s_ps.tile([128, 2 * TQ], f32, tag="s", name="pp1a")
        s2 = s_ps.tile([128, 2 * TQ], f32, tag="s", name="pp1b")
        slots = [s1[:, 0:TQ], s1[:, TQ:], s2[:, 0:TQ], s2[:, TQ:]]
        for n in range(4):
            nc.tensor.matmul(slots[n], wp[:, 128 * n : 128 * (n + 1)],
                             prhs0[lc][:], start=True, stop=False)
        for base in (0, 2):
            nc.tensor.matmul(slots[base], wp2[0:64, 128 * base : 128 * (base + 1)],
                             prhs1[lc][0:64, :], start=False, stop=True)
            nc.tensor.matmul(slots[base + 1],
                             wp2[64:128, 128 * (base + 1) : 128 * (base + 2)],
                             prhs1[lc][64:128, :], start=False, stop=True)
        for n in range(4):
            dst = osb_c[lc][:, TQ * n : TQ * (n + 1)]
            if n % 2:
                nc.scalar.copy(dst, slots[n])
            else:
                nc.vector.tensor_copy(dst, slots[n])
        nc.sync.dma_start(out_d[lc][:, 0 : 4 * TQ], osb_c[lc][:, 0 : 4 * TQ])
        emit_proj_pair(lc, 4)

    nc.compile()
    return nc


_PROG_CACHE = {}


def kernel(x, Wqkv, bqkv, Wproj, bproj):
    global LAST_RESULT
    x = np.asarray(x, dtype=np.float32)
    Wqkv = np.asarray(Wqkv, dtype=np.float32)
    bqkv = np.asarray(bqkv, dtype=np.float32)
    Wproj = np.asarray(Wproj, dtype=np.float32)
    bproj = np.asarray(bproj, dtype=np.float32)

    Wq, Wk, Wv = Wqkv[:, 0:C], Wqkv[:, C : 2 * C], Wqkv[:, 2 * C : 3 * C]
    bq, bk, bv = bqkv[0:C], bqkv[C : 2 * C], bqkv[2 * C : 3 * C]
    scale = 1.0 / np.sqrt(D)

    use_bias = bool(np.any(bq) or np.any(bk) or np.any(bv))
    if use_bias not in _PROG_CACHE:
        _PROG_CACHE[use_bias] = _build_program(use_bias)
    nc = _PROG_CACHE[use_bias]

    in_maps = []
    for c in range(N_CORES):
        b = c // (N_CORES // B)
        g = c % (N_CORES // B)
        hs = slice(DL * g, DL * (g + 1))       # this core's head-dim rows/cols

        # x^T packed per (chunk, k-tile): [NCH, 128, NKT*TQ]
        xt = np.ascontiguousarray(
            x[b].T.reshape(NKT, 128, NCH, TQ).transpose(2, 1, 0, 3)
        ).reshape(NCH, 128, NKT * TQ)
        xt1 = np.ones((1, TQ), np.float32)

        wq_loc = Wq[:, hs] * scale             # fold 1/sqrt(D) into q
        bq_loc = bq[hs] * scale
        wk_loc, bk_loc = Wk[:, hs], bk[hs]
        wv_loc, bv_loc = Wv[:, hs], bv[hs]

        wqk = np.zeros((C + 128, 2 * DL), np.float32)   # 7 k-tiles of 128
        wqk[0:C, 0:128] = wq_loc[:, 0:128]
        wqk[C, 0:128] = bq_loc[0:128]
        wqk[0:C, 128:256] = wk_loc[:, 0:128]
        wqk[C, 128:256] = bk_loc[0:128]
        wqk[0:C, 256:320] = wq_loc[:, 128:192]
        wqk[C, 256:320] = bq_loc[128:192]
        wqk[0:C, 320:384] = wk_loc[:, 128:192]
        wqk[C, 320:384] = bk_loc[128:192]
        # m-major: [m, j, 128] per partition row (production group m only
        # needs its own contiguous 896-column slice)
        wqk = np.concatenate(
            [np.ascontiguousarray(
                wqk[:, 128 * m : 128 * (m + 1)].reshape(7, 128, 128)
                .transpose(1, 0, 2)).reshape(128, 896)
             for m in range(3)], axis=1)

        wv_pad = np.zeros((C + 128, WV_W), np.float32)
        for h in range(H_LOC):
            c0 = DV * h
            wv_pad[0:C, c0 : c0 + D] = wv_loc[:, D * h : D * (h + 1)]
            wv_pad[C, c0 : c0 + D] = bv_loc[D * h : D * (h + 1)]
            wv_pad[C, c0 + D] = 1.0            # ones column -> softmax denom
        wv_pad = np.ascontiguousarray(
            wv_pad.reshape(7, 128, WV_W).transpose(1, 0, 2)).reshape(128, -1)

        wp = np.zeros((2, 128, C), np.float32)
        wp[0] = Wproj[DL * g : DL * g + 128, :]  # cast to bf16 below
        wp[1, 0:64] = Wproj[DL * g + 128 : DL * (g + 1), :]
        wp[1, 64:128] = wp[1, 0:64]              # dup for row-tiled proj half
        wp = np.ascontiguousarray(wp.transpose(1, 0, 2)).reshape(128, 2 * C)

        bf = ml_dtypes.bfloat16
        in_maps.append({"xt": xt.astype(bf), "xt1": xt1.astype(bf),
                        "wqk": wqk.astype(bf), "wv": wv_pad.astype(bf),
                        "wp": wp.astype(bf)})

    res = bass_utils.run_bass_kernel_spmd(nc, in_maps, core_ids=list(range(N_CORES)))
    LAST_RESULT = res

    out = np.zeros((B, T, C), np.float32)
    for c in range(N_CORES):
        b = c // (N_CORES // B)
        # outc [i, 128, n*512] -> [C, T] -> [T, C]
        outT = (res.results[c]["outc"].astype(np.float32)
                .reshape(NCH, 128, C // 128, TQ)
                .transpose(2, 1, 0, 3).reshape(C, T))
        out[b] += outT.T
    return out + bproj


if __name__ == "__main__":
    rng = np.random.default_rng(0)
    s = 1.0 / np.sqrt(C)
    ins = {
        "x": rng.standard_normal((B, T, C), dtype=np.float32),
        "Wqkv": rng.standard_normal((C, 3 * C), dtype=np.float32) * s,
        "bqkv": np.zeros(3 * C, np.float32),
        "Wproj": rng.standard_normal((C, C), dtype=np.float32) * s,
        "bproj": np.zeros(C, np.float32),
    }
    out = kernel(**ins)
    print("out", out.shape, out.dtype, float(np.abs(out).max()))
